# revision 16
# baseline (speedup 1.0000x reference)
"""GCN context-paper kernel for 8 trn2 NeuronCores (SPMD via bass/Tile).

Model (see reference): proj+LN -> 3x GCNConv(+self loops, sym-norm) with
GELU -> concat(4 hops) -> MLP(GELU) -> LN.

Sharding: nodes partitioned across 8 cores (2500/core, padded to 2560).
Per hop: each core computes Y = h @ W for its nodes, AllGathers Y (bf16),
then builds its nodes' aggregation with indirect row-gathers of Y plus
one-hot matmuls on the tensor engine (edge weights folded into the
one-hot values, self-loops folded into the edge list).

Layout strategy: activations are kept feature-major ("ct" tiles,
[128 feat, 2560 nodes]) which the scatter matmul produces directly and
all lhsT uses consume directly; only the proj output needs PE transposes.

DMA discipline: every DMA-queue instruction must end up with at most ONE
semaphore wait (hardware struct limit). Hence: DMA destinations in SBUF
are either fresh tiles or have engine-op (not DMA) prior writers; DMA
sources are external inputs or covered by dummy lane-warming DMAs
(collective output).
"""

import numpy as np
import ml_dtypes

import concourse.bass as bass
import concourse.bacc as bacc
import concourse.mybir as mybir
import concourse.tile as tile
from concourse.bass_utils import run_bass_kernel_spmd
from concourse.masks import make_identity

# problem constants (hardcoded per contract)
N, E, IN_F, H, HOPS = 20000, 100000, 1536, 768, 3
LN_EPS = 1e-5
NCORES = 8
NLOC = N // NCORES            # 2500 real nodes per core
P = 128
MT = 20                       # node tiles per core
NPAD = MT * P                 # 2560 padded nodes per core
HK = H // P                   # 6 feature tiles
HH = H // 2                   # feature half width (AllGather split)
HKH = HK // 2                 # feature tiles per half
INK = IN_F // P               # 12
CK = (HOPS + 1) * H // P      # 24 cat feature tiles
NSL = ((0, 512), (512, 256))  # N-dim slices for 768-wide outputs
OB = 8                        # chunks per one-hot load
GRP = 8                       # edge chunks per dma_gather
NWARM = 4                     # dummy lane-warming DMAs per collective

F32 = mybir.dt.float32
BF16 = mybir.dt.bfloat16
I32 = mybir.dt.int32
BF = ml_dtypes.bfloat16


# ---------------------------------------------------------------- host prep

def _prep(edge_index):
    """Host preprocessing: normalization, edge sorting, per-core chunk
    tables (gather indices + one-hot weight blocks)."""
    src = np.asarray(edge_index[0], dtype=np.int64)
    dst = np.asarray(edge_index[1], dtype=np.int64)
    deg = np.bincount(dst, minlength=N).astype(np.float64) + 1.0
    dis = 1.0 / np.sqrt(deg)

    # real edges only; self loops become a dedicated per-tile identity chunk
    alls, alld = src, dst
    w = (dis[alls] * dis[alld]).astype(np.float32)

    # global row in the AllGather output for each source node
    yg_row = (alls // NLOC) * NPAD + (alls % NLOC)

    # group edges by (core, dst tile)
    core = alld // NLOC
    loc = alld % NLOC
    t = loc // P
    d = loc % P  # local offset within dst tile
    counts = np.zeros((NCORES, MT), dtype=np.int64)
    np.add.at(counts, (core, t), 1)
    # chunk 0 of each tile = self loops (plain DMA from local Y); rest edges
    c_list = [1 + max(1, int(np.ceil(counts[:, tt].max() / P))) for tt in range(MT)]
    off = np.zeros(MT, dtype=np.int64)
    off[1:] = np.cumsum(c_list)[:-1]
    nch = int(sum(c_list))

    gidx = np.zeros((NCORES, P, nch), dtype=np.int32)
    oh = np.zeros((NCORES, nch * P, P), dtype=np.float32)

    # self chunks: diag(dis[d]^2) per (core, tile)
    for cc in range(NCORES):
        for tt in range(MT):
            nreal = min(P, NLOC - tt * P)
            gl = cc * NLOC + tt * P + np.arange(nreal)
            ch = off[tt]
            oh[cc, ch * P + np.arange(nreal), np.arange(nreal)] = (
                dis[gl] * dis[gl]
            )

    order = np.lexsort((alls, t, core))  # stable ordering by (core, tile)
    so_core, so_t, so_d = core[order], t[order], d[order]
    so_w, so_yg = w[order], yg_row[order]
    grp = so_core * MT + so_t
    start = np.zeros(NCORES * MT + 1, dtype=np.int64)
    np.add.at(start, grp + 1, 1)
    start = np.cumsum(start)
    pos = np.arange(len(order)) - start[grp]
    chunk = off[so_t] + 1 + pos // P
    row = pos % P
    gidx[so_core, row, chunk] = so_yg.astype(np.int32)
    oh[so_core, chunk * P + row, so_d] = so_w

    # int16 index stream for dma_gather: edge-chunk ids exclude self chunks
    n_self_before = np.zeros(nch, dtype=np.int64)
    for tt in range(MT):
        n_self_before[off[tt]:] += 0  # placeholder
    # chunk -> edge-chunk id: subtract #self chunks with index <= chunk
    selfmask = np.zeros(nch, dtype=np.int64)
    selfmask[off] = 1
    ech_of = np.cumsum(selfmask) - 1  # for self chunks: id of tile
    ech_map = np.arange(nch) - np.cumsum(selfmask)  # edge-chunk id (c>0)
    nech = nch - MT
    ni_tot = nech * P
    idx16 = np.zeros((NCORES, 128, ni_tot // 16), dtype=np.int16)
    e_ch = ech_map[chunk]  # edge-chunk id per sorted edge
    i_flat = e_ch * P + row
    p16 = i_flat % 16
    c16 = i_flat // 16
    for cc in range(NCORES):
        m = so_core == cc
        a = np.zeros((16, ni_tot // 16), np.int16)
        a[p16[m], c16[m]] = so_yg[m].astype(np.int16)
        idx16[cc] = np.tile(a, (8, 1))
    return nch, c_list, gidx, oh.astype(BF), idx16, nech


# --------------------------------------------------------------- bass build

def _build(nch, c_list, nech, stage=4, fake_ag=False):
    """Emit the SPMD Bass program. stage: 1=proj only, 2=+1 hop,
    3=+3 hops, 4=full (MLP+LN2). For stage<4 the output is the ct
    (feature-major) tiles of the last computed hop, [768, NPAD] f32."""
    nc = bacc.Bacc(
        "TRN2", target_bir_lowering=False, debug=False, num_devices=NCORES,
        num_swdge_queues=4,
    )
    dp = nc.declare_dram_parameter
    xT = dp("xT", [P, MT * IN_F], BF16, isOutput=False)
    projW = dp("projW", [IN_F, H], BF16, isOutput=False)
    gcnW = dp("gcnW", [HOPS * H, H], BF16, isOutput=False)
    w1 = dp("w1", [P, HK * (HOPS + 1) * H], BF16, isOutput=False)
    w2 = dp("w2", [H, H], BF16, isOutput=False)
    pbias = dp("pbias", [P, H], F32, isOutput=False)
    ln1g = dp("ln1g", [P, H], F32, isOutput=False)
    ln1b = dp("ln1b", [P, H], F32, isOutput=False)
    gbcol = dp("gbcol", [P, HOPS * HK], F32, isOutput=False)  # per-partition
    b1col = dp("b1col", [P, HK], F32, isOutput=False)
    b2 = dp("b2", [P, H], F32, isOutput=False)
    ln2g = dp("ln2g", [P, H], F32, isOutput=False)
    ln2b = dp("ln2b", [P, H], F32, isOutput=False)
    gidx = dp("gidx", [128, (nech * P) // 16], mybir.dt.int16, isOutput=False)
    ohw = dp("oh", [P, nch * P], BF16, isOutput=False)

    nhop = 0 if stage <= 1 else (1 if stage == 2 else HOPS)
    if stage >= 4:
        out = dp("out", [NPAD, H], F32, isOutput=True)
    else:
        out = dp("out", [H, NPAD], F32, isOutput=True)

    off = np.zeros(MT, dtype=np.int64)
    off[1:] = np.cumsum(c_list)[:-1]

    with tile.TileContext(nc) as tc:
        import contextlib

        with contextlib.ExitStack() as ctx:
            dram = ctx.enter_context(tc.tile_pool(name="dram", bufs=1, space="DRAM"))
            cat = ctx.enter_context(tc.tile_pool(name="cat", bufs=1))
            cst = ctx.enter_context(tc.tile_pool(name="cst", bufs=1))

            # persistent feature-major activation tiles
            ct = [cat.tile([P, NPAD], BF16, name=f"ct{i}") for i in range(CK)]
            # persistent slabs for 4 of the 6 w1 blocks (loaded after proj
            # so the MLP doesn't stall on its weight stream)
            w1pre = [cat.tile([P, CK, P], BF16, name=f"w1pre{f}")
                     for f in range(4)]

            idx_sb = cst.tile([128, (nech * P) // 16], mybir.dt.int16)
            gb_sb = cst.tile([P, HOPS * HK], F32)
            ident = cst.tile([P, P], BF16)
            make_identity(nc, ident[:])
            eps_t = cst.tile([P, 1], F32)
            nc.gpsimd.memset(eps_t[:], LN_EPS)

            # ---------------- proj + LN1 -> ct[0..5] (via PE transpose)
            with tc.tile_pool(name="proj", bufs=1) as pp, \
                    tc.tile_pool(name="psum_pj", bufs=1, space="PSUM") as psum:
                # first x slab first so the PE can start ASAP
                xs0 = pp.tile([P, INK, P], BF16, tag="xslab", bufs=3)
                nc.sync.dma_start(
                    out=xs0[:],
                    in_=xT[:, 0:IN_F].rearrange("p (k n) -> p k n", n=P),
                )
                pw = [pp.tile([P, H], BF16, name=f"pw{k}") for k in range(INK)]
                for k in range(INK):
                    nc.sync.dma_start(out=pw[k][:], in_=projW[k * P:(k + 1) * P, :])
                pb_sb = pp.tile([P, H], F32)
                l1g_sb = pp.tile([P, H], F32)
                l1b_sb = pp.tile([P, H], F32)
                nc.sync.dma_start(out=pb_sb[:], in_=pbias[:])
                nc.sync.dma_start(out=l1g_sb[:], in_=ln1g[:])
                nc.sync.dma_start(out=l1b_sb[:], in_=ln1b[:])

                for m in range(MT):
                    ms = slice(m * P, (m + 1) * P)
                    if m == 0:
                        xs = xs0
                    else:
                        xs = pp.tile([P, INK, P], BF16, tag="xslab", bufs=3)
                        nc.sync.dma_start(
                            out=xs[:],
                            in_=xT[:, m * IN_F:(m + 1) * IN_F].rearrange(
                                "p (k n) -> p k n", n=P
                            ),
                        )
                    ps = psum.tile([P, H], F32, tag="pj", bufs=2)
                    for n0, nn in NSL:
                        for k in range(INK):
                            nc.tensor.matmul(
                                out=ps[:, n0:n0 + nn],
                                lhsT=xs[:, k, :],
                                rhs=pw[k][:, n0:n0 + nn],
                                start=(k == 0),
                                stop=(k == INK - 1),
                            )
                    # LN1 over features (free dim), node-major.
                    # var = E[x^2] - mu^2 so the square (on ACT) overlaps the
                    # mean reduce (on DVE); Rsqrt + fused (x-mu)*rs cut DVE ops.
                    t0 = pp.tile([P, H], F32, tag="t0", bufs=2)
                    nc.vector.tensor_add(out=t0[:], in0=ps[:], in1=pb_sb[:])
                    mu = pp.tile([P, 1], F32, tag="mu", bufs=2)
                    nc.vector.reduce_sum(out=mu[:], in_=t0[:], axis=mybir.AxisListType.X)
                    sq = pp.tile([P, H], F32, tag="sq", bufs=2)
                    nc.scalar.activation(
                        out=sq[:], in_=t0[:],
                        func=mybir.ActivationFunctionType.Square,
                    )
                    var = pp.tile([P, 1], F32, tag="var", bufs=2)
                    nc.vector.reduce_sum(out=var[:], in_=sq[:], axis=mybir.AxisListType.X)
                    nc.scalar.mul(out=mu[:], in_=mu[:], mul=1.0 / H)
                    m2 = pp.tile([P, 1], F32, tag="m2", bufs=2)
                    nc.vector.tensor_mul(out=m2[:], in0=mu[:], in1=mu[:])
                    nc.vector.tensor_scalar(
                        out=var[:], in0=var[:], scalar1=1.0 / H, scalar2=None,
                        op0=mybir.AluOpType.mult,
                    )
                    nc.vector.tensor_sub(out=var[:], in0=var[:], in1=m2[:])
                    rs = pp.tile([P, 1], F32, tag="rs", bufs=2)
                    nc.scalar.activation(
                        out=rs[:], in_=var[:],
                        func=mybir.ActivationFunctionType.Sqrt,
                        bias=eps_t[:, :1],
                    )
                    nc.vector.reciprocal(out=rs[:], in_=rs[:])
                    nc.vector.tensor_scalar(
                        out=t0[:], in0=t0[:], scalar1=mu[:, :1], scalar2=rs[:, :1],
                        op0=mybir.AluOpType.subtract, op1=mybir.AluOpType.mult,
                    )
                    nc.vector.tensor_mul(out=t0[:], in0=t0[:], in1=l1g_sb[:])
                    h0 = pp.tile([P, H], BF16, tag="h0", bufs=2)
                    nc.vector.tensor_add(out=h0[:], in0=t0[:], in1=l1b_sb[:])
                    # transpose 6 blocks -> ct[f][:, m]
                    for f in range(HK):
                        tp = psum.tile([P, P], BF16, tag="tp", bufs=2)
                        nc.tensor.transpose(
                            out=tp[:], in_=h0[:, f * P:(f + 1) * P], identity=ident[:]
                        )
                        nc.vector.tensor_copy(out=ct[f][:, ms], in_=tp[:])

            # deferred loads: not needed until the first scatter pass / MLP,
            # so keep them off the critical startup DMA queue
            nc.sync.dma_start(out=idx_sb[:], in_=gidx[:])
            nc.sync.dma_start(out=gb_sb[:], in_=gbcol[:])
            for f in range(4):
                nc.sync.dma_start(
                    out=w1pre[f][:],
                    in_=w1[:, f * CK * P:(f + 1) * CK * P].rearrange(
                        "p (k n) -> p k n", n=P
                    ),
                )

            # ---------------- hops
            for k in range(nhop):
                hp = tc.tile_pool(name=f"hop{k}", bufs=1)
                with hp as hpool, \
                        tc.tile_pool(name=f"psum_h{k}", bufs=1, space="PSUM") as psum:
                    gw = [hpool.tile([P, H], BF16, name=f"gw{k}_{f}") for f in range(HK)]
                    for f in range(HK):
                        nc.sync.dma_start(
                            out=gw[f][:], in_=gcnW[k * H + f * P:k * H + (f + 1) * P, :]
                        )
                    ybig = hpool.tile([P, MT * H], BF16)
                    # feature-halved AllGather: gather/scatter of half 0
                    # overlaps the collective for half 1
                    agins = [
                        dram.tile([NPAD, HH], BF16, name=f"agin{k}_{h}")
                        for h in range(2)
                    ]
                    ygs = [
                        dram.tile([NCORES * NPAD, HH], BF16, addr_space="Shared",
                                  name=f"yg{k}_{h}")
                        for h in range(2)
                    ]
                    for m in range(MT):
                        ms = slice(m * P, (m + 1) * P)
                        ps = psum.tile([P, H], F32, tag="y", bufs=2)
                        for n0, nn in NSL:
                            for f in range(HK):
                                nc.tensor.matmul(
                                    out=ps[:, n0:n0 + nn],
                                    lhsT=ct[6 * k + f][:, ms],
                                    rhs=gw[f][:, n0:n0 + nn],
                                    start=(f == 0),
                                    stop=(f == HK - 1),
                                )
                        nc.vector.tensor_copy(out=ybig[:, m * H:(m + 1) * H], in_=ps[:])
                        # stream this tile's rows to DRAM immediately so the
                        # AllGather can start right after the last Y matmul
                        nc.sync.dma_start(
                            out=agins[0][m * P:(m + 1) * P, :],
                            in_=ybig[:, m * H:m * H + HH],
                        )
                        nc.scalar.dma_start(
                            out=agins[1][m * P:(m + 1) * P, :],
                            in_=ybig[:, m * H + HH:(m + 1) * H],
                        )
                    for h in range(2):
                        if fake_ag:
                            # timing-proxy only: local copy standing in for the
                            # AllGather (the sim's collective model is ~12x
                            # pessimistic for intra-chip groups)
                            nc.gpsimd.dma_start(out=ygs[h][0:NPAD, :], in_=agins[h][:])
                        else:
                            nc.gpsimd.collective_compute(
                                "AllGather",
                                mybir.AluOpType.bypass,
                                ins=[agins[h].opt()],
                                outs=[ygs[h].opt()],
                                replica_groups=[list(range(NCORES))],
                            )

                    # two scatter passes, one per feature half
                    nch_tot = int(sum(c_list))
                    for half in range(2):
                        yg = ygs[half]
                        # warm SWDGE lanes with 1-dep dummy reads of yg
                        for dlane in range(NWARM):
                            dmy = hpool.tile([2, 4], BF16,
                                             tag=f"dmy{half}_{dlane}", bufs=1)
                            nc.gpsimd.dma_start(
                                out=dmy[:], in_=yg[dlane * 2:dlane * 2 + 2, 0:4]
                            )
                        if k == 0 and half == 0:
                            # warm lanes on the idx region too (SBUF->SBUF tiny)
                            for dlane in range(NWARM):
                                dmi = hpool.tile([2, 1], I32, tag=f"dmi{dlane}",
                                                 bufs=1)
                                nc.gpsimd.dma_start(
                                    out=dmi[:], in_=idx_sb[dlane:dlane + 2, 0:1]
                                )
                        oh_tiles = {}
                        g_tiles = {}
                        ech = 0  # running edge-chunk id
                        for t in range(MT):
                            ts = slice(t * P, (t + 1) * P)
                            pa = psum.tile([P, HKH * P], F32,
                                           tag=f"sc{half}", bufs=2)
                            for c in range(c_list[t]):
                                ch = int(off[t]) + c
                                if c == 0:
                                    # self-loop chunk: local Y rows already in
                                    # SBUF node-major (ybig)
                                    gsl = ybig[:, t * H + half * HH:
                                               t * H + (half + 1) * HH]
                                else:
                                    gg, gj = ech // GRP, ech % GRP
                                    if gj == 0:
                                        ng = min(GRP, nech - gg * GRP)
                                        gt = hpool.tile(
                                            [P, ng, HH], BF16, tag="g",
                                            bufs=2, name=f"g{k}_{half}_{gg}",
                                        )
                                        nc.gpsimd.dma_gather(
                                            out_ap=gt[:],
                                            in_ap=yg[:],
                                            idxs_ap=idx_sb[
                                                :, gg * GRP * 8:(gg * GRP + ng) * 8
                                            ],
                                            num_idxs=ng * P,
                                            num_idxs_reg=ng * P,
                                            elem_size=HH,
                                            queue_num=gg % 4,
                                        )
                                        g_tiles[gg] = gt
                                    gsl = g_tiles[gg][:, gj, :]
                                    ech += 1
                                og, oj = ch // OB, ch % OB
                                if oj == 0:
                                    no = min(OB, nch_tot - og * OB)
                                    oh_t = hpool.tile(
                                        [P, no, P], BF16, tag="oh", bufs=3,
                                        name=f"oh{k}_{half}_{og}",
                                    )
                                    nc.sync.dma_start(
                                        out=oh_t[:],
                                        in_=ohw[
                                            :, og * OB * P:(og * OB + no) * P
                                        ].rearrange("p (c m) -> p c m", m=P),
                                    )
                                    oh_tiles[og] = oh_t
                                oh_t = oh_tiles[og]
                                first, last = (c == 0), (c == c_list[t] - 1)
                                for fi in range(HKH):
                                    nc.tensor.matmul(
                                        out=pa[:, fi * P:(fi + 1) * P],
                                        lhsT=gsl[:, fi * P:(fi + 1) * P],
                                        rhs=oh_t[:, oj, :],
                                        start=first and fi == 0,
                                        stop=last and fi == HKH - 1,
                                    )
                            for fi in range(HKH):
                                f = half * HKH + fi
                                nc.scalar.activation(
                                    out=ct[6 * (k + 1) + f][:, ts],
                                    in_=pa[:, fi * P:(fi + 1) * P],
                                    func=mybir.ActivationFunctionType.Gelu,
                                    bias=gb_sb[:, k * HK + f:k * HK + f + 1],
                                )

            if stage < 4:
                # dump last hop's ct tiles as [H, NPAD] f32
                with tc.tile_pool(name="dump", bufs=1) as dpool:
                    for f in range(HK):
                        df = dpool.tile([P, NPAD], F32, tag="df", bufs=2)
                        nc.vector.tensor_copy(out=df[:], in_=ct[6 * nhop + f][:])
                        nc.sync.dma_start(out=out[f * P:(f + 1) * P, :], in_=df[:])

            if stage >= 4:
                # ---------------- MLP + LN2
                with tc.tile_pool(name="mlp", bufs=1) as mp, \
                        tc.tile_pool(name="psum_mlp", bufs=1, space="PSUM") as psum:
                    w2t = [mp.tile([P, H], BF16, name=f"w2t{f}") for f in range(HK)]
                    for f in range(HK):
                        nc.sync.dma_start(out=w2t[f][:], in_=w2[f * P:(f + 1) * P, :])
                    b1_sb = mp.tile([P, HK], F32)
                    nc.sync.dma_start(out=b1_sb[:], in_=b1col[:])
                    b2_sb = mp.tile([P, H], F32)
                    l2g_sb = mp.tile([P, H], F32)
                    l2b_sb = mp.tile([P, H], F32)
                    nc.sync.dma_start(out=b2_sb[:], in_=b2[:])
                    nc.sync.dma_start(out=l2g_sb[:], in_=ln2g[:])
                    nc.sync.dma_start(out=l2b_sb[:], in_=ln2b[:])
                    # w1: blocks 0-3 were preloaded persistently after proj;
                    # blocks 4-5 stream here, overlapped with the first
                    # chunk's matmuls on blocks 0-3
                    w1sb = w1pre + [
                        mp.tile([P, CK, P], BF16, name=f"w1sb{f}")
                        for f in range(4, HK)
                    ]
                    for n in range(5):  # 512-wide node chunks
                        ns = slice(n * 512, (n + 1) * 512)
                        zt = [
                            mp.tile([P, 512], BF16, tag=f"zt{f}", bufs=2, name=f"zt{f}")
                            for f in range(HK)
                        ]
                        for f in range(HK):
                            if n == 0 and f >= 4:
                                nc.sync.dma_start(
                                    out=w1sb[f][:],
                                    in_=w1[:, f * CK * P:(f + 1) * CK * P].rearrange(
                                        "p (k n) -> p k n", n=P
                                    ),
                                )
                            pz = psum.tile([P, 512], F32, tag="z", bufs=2)
                            for kk in range(CK):
                                nc.tensor.matmul(
                                    out=pz[:],
                                    lhsT=w1sb[f][:, kk, :],
                                    rhs=ct[kk][:, ns],
                                    start=(kk == 0),
                                    stop=(kk == CK - 1),
                                )
                            nc.scalar.activation(
                                out=zt[f][:],
                                in_=pz[:],
                                func=mybir.ActivationFunctionType.Gelu,
                                bias=b1_sb[:, f:f + 1],
                            )
                        for mm in range(4):
                            m = n * 4 + mm
                            po = psum.tile([P, H], F32, tag="o", bufs=2)
                            for n0, nn in NSL:
                                for f in range(HK):
                                    nc.tensor.matmul(
                                        out=po[:, n0:n0 + nn],
                                        lhsT=zt[f][:, mm * P:(mm + 1) * P],
                                        rhs=w2t[f][:, n0:n0 + nn],
                                        start=(f == 0),
                                        stop=(f == HK - 1),
                                    )
                            t0 = mp.tile([P, H], F32, tag="t0", bufs=2)
                            nc.vector.tensor_add(out=t0[:], in0=po[:], in1=b2_sb[:])
                            mu = mp.tile([P, 1], F32, tag="mu", bufs=2)
                            nc.vector.reduce_sum(
                                out=mu[:], in_=t0[:], axis=mybir.AxisListType.X
                            )
                            sq = mp.tile([P, H], F32, tag="sq", bufs=2)
                            nc.scalar.activation(
                                out=sq[:], in_=t0[:],
                                func=mybir.ActivationFunctionType.Square,
                            )
                            var = mp.tile([P, 1], F32, tag="var", bufs=2)
                            nc.vector.reduce_sum(
                                out=var[:], in_=sq[:], axis=mybir.AxisListType.X
                            )
                            nc.scalar.mul(out=mu[:], in_=mu[:], mul=1.0 / H)
                            m2 = mp.tile([P, 1], F32, tag="m2", bufs=2)
                            nc.vector.tensor_mul(out=m2[:], in0=mu[:], in1=mu[:])
                            nc.vector.tensor_scalar(
                                out=var[:], in0=var[:], scalar1=1.0 / H,
                                scalar2=None, op0=mybir.AluOpType.mult,
                            )
                            nc.vector.tensor_sub(out=var[:], in0=var[:], in1=m2[:])
                            rs = mp.tile([P, 1], F32, tag="rs", bufs=2)
                            nc.scalar.activation(
                                out=rs[:], in_=var[:],
                                func=mybir.ActivationFunctionType.Sqrt,
                                bias=eps_t[:, :1],
                            )
                            nc.vector.reciprocal(out=rs[:], in_=rs[:])
                            nc.vector.tensor_scalar(
                                out=t0[:], in0=t0[:], scalar1=mu[:, :1],
                                scalar2=rs[:, :1],
                                op0=mybir.AluOpType.subtract,
                                op1=mybir.AluOpType.mult,
                            )
                            nc.vector.tensor_mul(out=t0[:], in0=t0[:], in1=l2g_sb[:])
                            ot = mp.tile([P, H], F32, tag="ot", bufs=2)
                            nc.vector.tensor_add(out=ot[:], in0=t0[:], in1=l2b_sb[:])
                            nc.sync.dma_start(out=out[m * P:(m + 1) * P, :], in_=ot[:])
    nc.compile()
    return nc


def check_waits(nc, limit=1):
    """Return list of DMA-queue instructions exceeding the wait limit."""
    bad = []
    for f in nc.m.functions:
        for bb in f.blocks:
            for ins in bb.instructions:
                tn = type(ins).__name__
                if tn not in ("InstDMACopy", "InstDmaTransposeAnt"):
                    continue
                si = ins.sync_info
                if len(si.on_wait) > limit:
                    bad.append(
                        (ins.name, tn, str(ins.engine),
                         [(w.ant_name, w.wait_value) for w in si.on_wait])
                    )
    return bad


# ------------------------------------------------------------- entry point

def _in_maps(inputs, nch, gidx, oh, idx16):
    x = np.asarray(inputs["x"], dtype=np.float32)
    bcast = lambda v: np.broadcast_to(
        np.asarray(v, np.float32), (P, H)
    ).copy()
    gb = np.asarray(inputs["gcn_b"], np.float32)  # [HOPS, H]
    gbcol = np.zeros((P, HOPS * HK), np.float32)
    for k in range(HOPS):
        for f in range(HK):
            gbcol[:, k * HK + f] = gb[k, f * P:(f + 1) * P]
    b1 = np.asarray(inputs["mlp_b1"], np.float32)
    b1col = np.zeros((P, HK), np.float32)
    for f in range(HK):
        b1col[:, f] = b1[f * P:(f + 1) * P]
    w1 = np.asarray(inputs["mlp_w1"], np.float32)  # [3072, 768]
    w1p = np.zeros((P, HK * (HOPS + 1) * H), np.float32)
    for f in range(HK):
        blk = w1[:, f * P:(f + 1) * P]  # [3072, 128]
        w1p[:, f * CK * P:(f + 1) * CK * P] = (
            blk.reshape(CK, P, P).transpose(1, 0, 2).reshape(P, CK * P)
        )
    common = {
        "projW": np.asarray(inputs["proj_w"], np.float32).astype(BF),
        "gcnW": np.asarray(inputs["gcn_w"], np.float32).reshape(HOPS * H, H).astype(BF),
        "w1": w1p.astype(BF),
        "w2": np.asarray(inputs["mlp_w2"], np.float32).astype(BF),
        "pbias": bcast(inputs["proj_b"]),
        "ln1g": bcast(inputs["ln1_g"]),
        "ln1b": bcast(inputs["ln1_b"]),
        "gbcol": gbcol,
        "b1col": b1col,
        "b2": bcast(inputs["mlp_b2"]),
        "ln2g": bcast(inputs["ln2_g"]),
        "ln2b": bcast(inputs["ln2_b"]),
    }
    maps = []
    for c in range(NCORES):
        xc = np.zeros((NPAD, IN_F), np.float32)
        xc[:NLOC] = x[c * NLOC:(c + 1) * NLOC]
        # pack: xp[p, m*IN_F + k*128 + n] = x[m*128+n, k*128+p]
        xp = (
            xc.reshape(MT, P, INK, P)      # [m, n, k, p]
            .transpose(3, 0, 2, 1)          # [p, m, k, n]
            .reshape(P, MT * IN_F)
        )
        ohp = (
            oh[c].astype(np.float32).reshape(-1, P, P)  # [ch, p, d]
            .transpose(1, 0, 2)                          # [p, ch, d]
            .reshape(P, -1)
        )
        m = dict(common)
        m["xT"] = xp.astype(BF)
        m["gidx"] = idx16[c]
        m["oh"] = ohp.astype(BF)
        maps.append(m)
    return maps


def kernel(**inputs):
    nch, c_list, gidx, oh, idx16, nech = _prep(np.asarray(inputs["edge_index"]))
    nc = _build(nch, c_list, nech, stage=4)
    maps = _in_maps(inputs, nch, gidx, oh, idx16)
    res = run_bass_kernel_spmd(nc, maps, list(range(NCORES)))
    outs = [res.results[c]["out"][:NLOC] for c in range(NCORES)]
    return np.concatenate(outs, axis=0).astype(np.float32)



# revision 23
# speedup vs baseline: 1.1369x; 1.1369x over previous
"""GCN context-paper kernel for 8 trn2 NeuronCores (SPMD via bass/Tile).

Model (see reference): proj+LN -> 3x GCNConv(+self loops, sym-norm) with
GELU -> concat(4 hops) -> MLP(GELU) -> LN.

Sharding: nodes partitioned across 8 cores (2500/core, padded to 2560).
Per hop: each core computes Y = h @ W for its nodes, AllGathers Y (bf16),
then builds its nodes' aggregation with indirect row-gathers of Y plus
one-hot matmuls on the tensor engine (edge weights folded into the
one-hot values, self-loops folded into the edge list).

Layout strategy: activations are kept feature-major ("ct" tiles,
[128 feat, 2560 nodes]) which the scatter matmul produces directly and
all lhsT uses consume directly; only the proj output needs PE transposes.

DMA discipline: every DMA-queue instruction must end up with at most ONE
semaphore wait (hardware struct limit). Hence: DMA destinations in SBUF
are either fresh tiles or have engine-op (not DMA) prior writers; DMA
sources are external inputs or covered by dummy lane-warming DMAs
(collective output).
"""

import numpy as np
import ml_dtypes

import concourse.bass as bass
import concourse.bacc as bacc
import concourse.mybir as mybir
import concourse.tile as tile
from concourse.bass_utils import run_bass_kernel_spmd
from concourse.masks import make_identity

# problem constants (hardcoded per contract)
N, E, IN_F, H, HOPS = 20000, 100000, 1536, 768, 3
LN_EPS = 1e-5
NCORES = 8
NLOC = N // NCORES            # 2500 real nodes per core
P = 128
MT = 20                       # node tiles per core
NPAD = MT * P                 # 2560 padded nodes per core
HK = H // P                   # 6 feature tiles
HH = H // 2                   # feature half width (AllGather split)
HKH = HK // 2                 # feature tiles per half
INK = IN_F // P               # 12
CK = (HOPS + 1) * H // P      # 24 cat feature tiles
NSL = ((0, 512), (512, 256))  # N-dim slices for 768-wide outputs
OB = 8                        # chunks per one-hot load
GRP = 8                       # edge chunks per dma_gather
NWARM = 4                     # dummy lane-warming DMAs per collective

# flat offsets into the consolidated bf16 weight tensor (elements)
PROJ_OFF = 0
GCN_OFF = PROJ_OFF + IN_F * H
W2_OFF = GCN_OFF + HOPS * H * H
W1_OFF = W2_OFF + H * H
WBTOT = W1_OFF + (HOPS + 1) * H * H
# flat offsets into the consolidated f32 table section (per-partition cols)
PB_OFF, L1G_OFF, L1B_OFF = 0, H, 2 * H
B2_OFF, L2G_OFF, L2B_OFF = 3 * H, 4 * H, 5 * H
GB_OFF = 6 * H
B1_OFF = GB_OFF + HOPS * HK
FTOT = B1_OFF + HK


def _pad16(x):
    return (x + 15) // 16 * 16


def _xc_layout(nch, nech):
    """bf16-column offsets of the sections in the per-core xc tensor."""
    xt_off = 0
    oh_off = _pad16(xt_off + MT * IN_F)
    idx_off = _pad16(oh_off + nch * P)
    wf_off = _pad16(idx_off + (nech * P) // 16)
    tot = _pad16(wf_off + 2 * FTOT)
    return xt_off, oh_off, idx_off, wf_off, tot

F32 = mybir.dt.float32
BF16 = mybir.dt.bfloat16
I32 = mybir.dt.int32
BF = ml_dtypes.bfloat16


# ---------------------------------------------------------------- host prep

def _prep(edge_index):
    """Host preprocessing: normalization, edge sorting, per-core chunk
    tables (gather indices + one-hot weight blocks)."""
    src = np.asarray(edge_index[0], dtype=np.int64)
    dst = np.asarray(edge_index[1], dtype=np.int64)
    deg = np.bincount(dst, minlength=N).astype(np.float64) + 1.0
    dis = 1.0 / np.sqrt(deg)

    # real edges only; self loops become a dedicated per-tile identity chunk
    alls, alld = src, dst
    w = (dis[alls] * dis[alld]).astype(np.float32)

    # global row in the AllGather output for each source node
    yg_row = (alls // NLOC) * NPAD + (alls % NLOC)

    # group edges by (core, dst tile)
    core = alld // NLOC
    loc = alld % NLOC
    t = loc // P
    d = loc % P  # local offset within dst tile
    counts = np.zeros((NCORES, MT), dtype=np.int64)
    np.add.at(counts, (core, t), 1)
    # chunk 0 of each tile = self loops (plain DMA from local Y); rest edges
    c_list = [1 + max(1, int(np.ceil(counts[:, tt].max() / P))) for tt in range(MT)]
    off = np.zeros(MT, dtype=np.int64)
    off[1:] = np.cumsum(c_list)[:-1]
    nch = int(sum(c_list))

    gidx = np.zeros((NCORES, P, nch), dtype=np.int32)
    oh = np.zeros((NCORES, nch * P, P), dtype=np.float32)

    # self chunks: diag(dis[d]^2) per (core, tile)
    for cc in range(NCORES):
        for tt in range(MT):
            nreal = min(P, NLOC - tt * P)
            gl = cc * NLOC + tt * P + np.arange(nreal)
            ch = off[tt]
            oh[cc, ch * P + np.arange(nreal), np.arange(nreal)] = (
                dis[gl] * dis[gl]
            )

    order = np.lexsort((alls, t, core))  # stable ordering by (core, tile)
    so_core, so_t, so_d = core[order], t[order], d[order]
    so_w, so_yg = w[order], yg_row[order]
    grp = so_core * MT + so_t
    start = np.zeros(NCORES * MT + 1, dtype=np.int64)
    np.add.at(start, grp + 1, 1)
    start = np.cumsum(start)
    pos = np.arange(len(order)) - start[grp]
    chunk = off[so_t] + 1 + pos // P
    row = pos % P
    gidx[so_core, row, chunk] = so_yg.astype(np.int32)
    oh[so_core, chunk * P + row, so_d] = so_w

    # int16 index stream for dma_gather: edge-chunk ids exclude self chunks
    n_self_before = np.zeros(nch, dtype=np.int64)
    for tt in range(MT):
        n_self_before[off[tt]:] += 0  # placeholder
    # chunk -> edge-chunk id: subtract #self chunks with index <= chunk
    selfmask = np.zeros(nch, dtype=np.int64)
    selfmask[off] = 1
    ech_of = np.cumsum(selfmask) - 1  # for self chunks: id of tile
    ech_map = np.arange(nch) - np.cumsum(selfmask)  # edge-chunk id (c>0)
    nech = nch - MT
    ni_tot = nech * P
    idx16 = np.zeros((NCORES, 128, ni_tot // 16), dtype=np.int16)
    e_ch = ech_map[chunk]  # edge-chunk id per sorted edge
    i_flat = e_ch * P + row
    p16 = i_flat % 16
    c16 = i_flat // 16
    for cc in range(NCORES):
        m = so_core == cc
        a = np.zeros((16, ni_tot // 16), np.int16)
        a[p16[m], c16[m]] = so_yg[m].astype(np.int16)
        idx16[cc] = np.tile(a, (8, 1))
    return nch, c_list, gidx, oh.astype(BF), idx16, nech


# --------------------------------------------------------------- bass build

def _build(nch, c_list, nech, stage=4, fake_ag=False, split_ag=False):
    """Emit the SPMD Bass program. stage: 1=proj only, 2=+1 hop,
    3=+3 hops, 4=full (MLP+LN2). For stage<4 the output is the ct
    (feature-major) tiles of the last computed hop, [768, NPAD] f32."""
    nc = bacc.Bacc(
        "TRN2", target_bir_lowering=False, debug=False, num_devices=NCORES,
        num_swdge_queues=4,
    )
    dp = nc.declare_dram_parameter
    # consolidated operands: fewer kernel params -> much lower per-exec
    # dispatch cost through the PJRT tunnel (~40 us/operand measured).
    # xc = [x^T | one-hot tables | gather indices (bitcast) | f32 tables
    # (bitcast)], all per-core; wb = shared flat bf16 weights.
    XT_OFF, OH_OFF, IDX_OFF, WF_OFF, XCTOT = _xc_layout(nch, nech)
    xc = dp("xc", [P, XCTOT], BF16, isOutput=False)
    wb = dp("wb", [1, WBTOT], BF16, isOutput=False)

    def wb2d(off, rows, cols):
        # contiguous [rows, cols] block at flat offset `off` in wb
        return wb[0:1, off:off + rows * cols].rearrange(
            "o (p h) -> (o p) h", p=rows
        )

    def wfap(off, width):
        # f32 view into the per-core xc tensor
        return xc[:, WF_OFF + 2 * off:WF_OFF + 2 * (off + width)].bitcast(F32)

    nhop = 0 if stage <= 1 else (1 if stage == 2 else HOPS)
    if stage >= 4:
        out = dp("out", [NPAD, H], F32, isOutput=True)
    else:
        out = dp("out", [H, NPAD], F32, isOutput=True)

    off = np.zeros(MT, dtype=np.int64)
    off[1:] = np.cumsum(c_list)[:-1]

    with tile.TileContext(nc) as tc:
        import contextlib

        with contextlib.ExitStack() as ctx:
            dram = ctx.enter_context(tc.tile_pool(name="dram", bufs=1, space="DRAM"))
            cat = ctx.enter_context(tc.tile_pool(name="cat", bufs=1))
            cst = ctx.enter_context(tc.tile_pool(name="cst", bufs=1))

            # persistent feature-major activation tiles
            ct = [cat.tile([P, NPAD], BF16, name=f"ct{i}") for i in range(CK)]
            # persistent slabs for 4 of the 6 w1 blocks (loaded after proj
            # so the MLP doesn't stall on its weight stream); only fits
            # when the halved gather tiles free enough SBUF
            npre = 4 if split_ag else 0
            w1pre = [cat.tile([P, CK, P], BF16, name=f"w1pre{f}")
                     for f in range(npre)]

            idx_sb = cst.tile([128, (nech * P) // 16], mybir.dt.int16)
            gb_sb = cst.tile([P, HOPS * HK], F32)
            ident = cst.tile([P, P], BF16)
            make_identity(nc, ident[:])
            eps_t = cst.tile([P, 1], F32)
            nc.gpsimd.memset(eps_t[:], LN_EPS)

            # ---------------- proj + LN1 -> ct[0..5] (via PE transpose)
            with tc.tile_pool(name="proj", bufs=1) as pp, \
                    tc.tile_pool(name="psum_pj", bufs=1, space="PSUM") as psum:
                # first x slab first so the PE can start ASAP
                xs0 = pp.tile([P, INK, P], BF16, tag="xslab", bufs=3)
                nc.sync.dma_start(
                    out=xs0[:],
                    in_=xc[:, XT_OFF:XT_OFF + IN_F].rearrange(
                        "p (k n) -> p k n", n=P
                    ),
                )
                pw = [pp.tile([P, H], BF16, name=f"pw{k}") for k in range(INK)]
                for k in range(INK):
                    nc.sync.dma_start(
                        out=pw[k][:], in_=wb2d(PROJ_OFF + k * P * H, P, H)
                    )
                lt1 = pp.tile([P, 3 * H], F32)
                nc.sync.dma_start(out=lt1[:], in_=wfap(PB_OFF, 3 * H))
                pb_sb = lt1[:, 0:H]
                l1g_sb = lt1[:, H:2 * H]
                l1b_sb = lt1[:, 2 * H:3 * H]

                for m in range(MT):
                    ms = slice(m * P, (m + 1) * P)
                    if m == 0:
                        xs = xs0
                    else:
                        xs = pp.tile([P, INK, P], BF16, tag="xslab", bufs=3)
                        nc.sync.dma_start(
                            out=xs[:],
                            in_=xc[:, XT_OFF + m * IN_F:
                                   XT_OFF + (m + 1) * IN_F].rearrange(
                                "p (k n) -> p k n", n=P
                            ),
                        )
                    ps = psum.tile([P, H], F32, tag="pj", bufs=2)
                    for n0, nn in NSL:
                        for k in range(INK):
                            nc.tensor.matmul(
                                out=ps[:, n0:n0 + nn],
                                lhsT=xs[:, k, :],
                                rhs=pw[k][:, n0:n0 + nn],
                                start=(k == 0),
                                stop=(k == INK - 1),
                            )
                    # LN1 over features (free dim), node-major.
                    # var = E[x^2] - mu^2 so the square (on ACT) overlaps the
                    # mean reduce (on DVE); Rsqrt + fused (x-mu)*rs cut DVE ops.
                    t0 = pp.tile([P, H], F32, tag="t0", bufs=2)
                    nc.vector.tensor_add(out=t0[:], in0=ps[:], in1=pb_sb)
                    mu = pp.tile([P, 1], F32, tag="mu", bufs=2)
                    nc.vector.reduce_sum(out=mu[:], in_=t0[:], axis=mybir.AxisListType.X)
                    sq = pp.tile([P, H], F32, tag="sq", bufs=2)
                    nc.scalar.activation(
                        out=sq[:], in_=t0[:],
                        func=mybir.ActivationFunctionType.Square,
                    )
                    var = pp.tile([P, 1], F32, tag="var", bufs=2)
                    nc.vector.reduce_sum(out=var[:], in_=sq[:], axis=mybir.AxisListType.X)
                    nc.scalar.mul(out=mu[:], in_=mu[:], mul=1.0 / H)
                    m2 = pp.tile([P, 1], F32, tag="m2", bufs=2)
                    nc.vector.tensor_mul(out=m2[:], in0=mu[:], in1=mu[:])
                    nc.vector.tensor_scalar(
                        out=var[:], in0=var[:], scalar1=1.0 / H, scalar2=None,
                        op0=mybir.AluOpType.mult,
                    )
                    nc.vector.tensor_sub(out=var[:], in0=var[:], in1=m2[:])
                    rs = pp.tile([P, 1], F32, tag="rs", bufs=2)
                    nc.scalar.activation(
                        out=rs[:], in_=var[:],
                        func=mybir.ActivationFunctionType.Sqrt,
                        bias=eps_t[:, :1],
                    )
                    nc.vector.reciprocal(out=rs[:], in_=rs[:])
                    nc.vector.tensor_scalar(
                        out=t0[:], in0=t0[:], scalar1=mu[:, :1], scalar2=rs[:, :1],
                        op0=mybir.AluOpType.subtract, op1=mybir.AluOpType.mult,
                    )
                    nc.vector.tensor_mul(out=t0[:], in0=t0[:], in1=l1g_sb)
                    h0 = pp.tile([P, H], BF16, tag="h0", bufs=2)
                    nc.vector.tensor_add(out=h0[:], in0=t0[:], in1=l1b_sb)
                    # transpose 6 blocks -> ct[f][:, m]
                    for f in range(HK):
                        tp = psum.tile([P, P], BF16, tag="tp", bufs=2)
                        nc.tensor.transpose(
                            out=tp[:], in_=h0[:, f * P:(f + 1) * P], identity=ident[:]
                        )
                        nc.vector.tensor_copy(out=ct[f][:, ms], in_=tp[:])

            # deferred loads: not needed until the first scatter pass / MLP,
            # so keep them off the critical startup DMA queue
            nc.sync.dma_start(
                out=idx_sb[:],
                in_=xc[:, IDX_OFF:IDX_OFF + (nech * P) // 16].bitcast(
                    mybir.dt.int16
                ),
            )
            nc.sync.dma_start(out=gb_sb[:], in_=wfap(GB_OFF, HOPS * HK))
            for f in range(npre):
                nc.sync.dma_start(
                    out=w1pre[f][:],
                    in_=wb[0:1, W1_OFF + f * P * CK * P:
                           W1_OFF + (f + 1) * P * CK * P].rearrange(
                        "o (p k n) -> (o p) k n", p=P, n=P
                    ),
                )

            # ---------------- hops
            for k in range(nhop):
                hp = tc.tile_pool(name=f"hop{k}", bufs=1)
                with hp as hpool, \
                        tc.tile_pool(name=f"psum_h{k}", bufs=1, space="PSUM") as psum:
                    gw = [hpool.tile([P, H], BF16, name=f"gw{k}_{f}") for f in range(HK)]
                    for f in range(HK):
                        nc.sync.dma_start(
                            out=gw[f][:],
                            in_=wb2d(GCN_OFF + (k * H + f * P) * H, P, H),
                        )
                    ybig = hpool.tile([P, MT * H], BF16)
                    # feature-halved AllGather: gather/scatter of half 0
                    # overlaps the collective for half 1 (split_ag=True);
                    # otherwise one full-width AllGather
                    nhalf = 2 if split_ag else 1
                    hw_ = HH if split_ag else H
                    agins = [
                        dram.tile([NPAD, hw_], BF16, name=f"agin{k}_{h}")
                        for h in range(nhalf)
                    ]
                    ygs = [
                        dram.tile([NCORES * NPAD, hw_], BF16, addr_space="Shared",
                                  name=f"yg{k}_{h}")
                        for h in range(nhalf)
                    ]
                    for m in range(MT):
                        ms = slice(m * P, (m + 1) * P)
                        ps = psum.tile([P, H], F32, tag="y", bufs=2)
                        for n0, nn in NSL:
                            for f in range(HK):
                                nc.tensor.matmul(
                                    out=ps[:, n0:n0 + nn],
                                    lhsT=ct[6 * k + f][:, ms],
                                    rhs=gw[f][:, n0:n0 + nn],
                                    start=(f == 0),
                                    stop=(f == HK - 1),
                                )
                        nc.vector.tensor_copy(out=ybig[:, m * H:(m + 1) * H], in_=ps[:])
                        # stream this tile's rows to DRAM immediately so the
                        # AllGather can start right after the last Y matmul
                        nc.sync.dma_start(
                            out=agins[0][m * P:(m + 1) * P, :],
                            in_=ybig[:, m * H:m * H + hw_],
                        )
                        if split_ag:
                            nc.scalar.dma_start(
                                out=agins[1][m * P:(m + 1) * P, :],
                                in_=ybig[:, m * H + HH:(m + 1) * H],
                            )
                    for h in range(nhalf):
                        if fake_ag:
                            # timing-proxy only: local copy standing in for the
                            # AllGather (the sim's collective model is ~12x
                            # pessimistic for intra-chip groups)
                            nc.gpsimd.dma_start(out=ygs[h][0:NPAD, :], in_=agins[h][:])
                        else:
                            nc.gpsimd.collective_compute(
                                "AllGather",
                                mybir.AluOpType.bypass,
                                ins=[agins[h].opt()],
                                outs=[ygs[h].opt()],
                                replica_groups=[list(range(NCORES))],
                            )

                    # scatter passes, one per feature half
                    nch_tot = int(sum(c_list))
                    for half in range(nhalf):
                        yg = ygs[half]
                        # warm SWDGE lanes with 1-dep dummy reads of yg
                        for dlane in range(NWARM):
                            dmy = hpool.tile([2, 4], BF16,
                                             tag=f"dmy{half}_{dlane}", bufs=1)
                            nc.gpsimd.dma_start(
                                out=dmy[:], in_=yg[dlane * 2:dlane * 2 + 2, 0:4]
                            )
                        if k == 0 and half == 0:
                            # warm lanes on the idx region too (SBUF->SBUF tiny)
                            for dlane in range(NWARM):
                                dmi = hpool.tile([2, 1], I32, tag=f"dmi{dlane}",
                                                 bufs=1)
                                nc.gpsimd.dma_start(
                                    out=dmi[:], in_=idx_sb[dlane:dlane + 2, 0:1]
                                )
                        oh_tiles = {}
                        g_tiles = {}
                        ech = 0  # running edge-chunk id
                        for t in range(MT):
                            ts = slice(t * P, (t + 1) * P)
                            npa = HKH if split_ag else HK
                            if split_ag:
                                pa = psum.tile([P, HKH * P], F32,
                                               tag=f"sc{half}", bufs=2)
                                pb_ = None
                            else:
                                pa = psum.tile([P, 512], F32, tag="sca", bufs=2)
                                pb_ = psum.tile([P, 256], F32, tag="scb", bufs=2)

                            def _dst(fi):
                                if pb_ is None or fi < 4:
                                    return pa[:, (fi % 4) * P:(fi % 4 + 1) * P]
                                return pb_[:, (fi - 4) * P:(fi - 3) * P]

                            starts = (0,) if pb_ is None else (0, 4)
                            stops = (npa - 1,) if pb_ is None else (3, 5)
                            for c in range(c_list[t]):
                                ch = int(off[t]) + c
                                if c == 0:
                                    # self-loop chunk: local Y rows already in
                                    # SBUF node-major (ybig)
                                    gsl = ybig[:, t * H + half * HH:
                                               t * H + half * HH + hw_]
                                else:
                                    gg, gj = ech // GRP, ech % GRP
                                    if gj == 0:
                                        ng = min(GRP, nech - gg * GRP)
                                        gt = hpool.tile(
                                            [P, ng, hw_], BF16, tag="g",
                                            bufs=2, name=f"g{k}_{half}_{gg}",
                                        )
                                        nc.gpsimd.dma_gather(
                                            out_ap=gt[:],
                                            in_ap=yg[:],
                                            idxs_ap=idx_sb[
                                                :, gg * GRP * 8:(gg * GRP + ng) * 8
                                            ],
                                            num_idxs=ng * P,
                                            num_idxs_reg=ng * P,
                                            elem_size=hw_,
                                            queue_num=gg % 4,
                                        )
                                        g_tiles[gg] = gt
                                    gsl = g_tiles[gg][:, gj, :]
                                    ech += 1
                                og, oj = ch // OB, ch % OB
                                if oj == 0:
                                    no = min(OB, nch_tot - og * OB)
                                    oh_t = hpool.tile(
                                        [P, no, P], BF16, tag="oh", bufs=3,
                                        name=f"oh{k}_{half}_{og}",
                                    )
                                    nc.sync.dma_start(
                                        out=oh_t[:],
                                        in_=xc[
                                            :, OH_OFF + og * OB * P:
                                            OH_OFF + (og * OB + no) * P
                                        ].rearrange("p (c m) -> p c m", m=P),
                                    )
                                    oh_tiles[og] = oh_t
                                oh_t = oh_tiles[og]
                                first, last = (c == 0), (c == c_list[t] - 1)
                                for fi in range(npa):
                                    nc.tensor.matmul(
                                        out=_dst(fi),
                                        lhsT=gsl[:, fi * P:(fi + 1) * P],
                                        rhs=oh_t[:, oj, :],
                                        start=first and fi in starts,
                                        stop=last and fi in stops,
                                    )
                            for fi in range(npa):
                                f = half * HKH + fi
                                nc.scalar.activation(
                                    out=ct[6 * (k + 1) + f][:, ts],
                                    in_=_dst(fi),
                                    func=mybir.ActivationFunctionType.Gelu,
                                    bias=gb_sb[:, k * HK + f:k * HK + f + 1],
                                )

            if stage < 4:
                # dump last hop's ct tiles as [H, NPAD] f32
                with tc.tile_pool(name="dump", bufs=1) as dpool:
                    for f in range(HK):
                        df = dpool.tile([P, NPAD], F32, tag="df", bufs=2)
                        nc.vector.tensor_copy(out=df[:], in_=ct[6 * nhop + f][:])
                        nc.sync.dma_start(out=out[f * P:(f + 1) * P, :], in_=df[:])

            if stage >= 4:
                # ---------------- MLP + LN2
                with tc.tile_pool(name="mlp", bufs=1) as mp, \
                        tc.tile_pool(name="psum_mlp", bufs=1, space="PSUM") as psum:
                    w2t = [mp.tile([P, H], BF16, name=f"w2t{f}") for f in range(HK)]
                    for f in range(HK):
                        nc.sync.dma_start(
                            out=w2t[f][:], in_=wb2d(W2_OFF + f * P * H, P, H)
                        )
                    b1_sb = mp.tile([P, HK], F32)
                    nc.sync.dma_start(out=b1_sb[:], in_=wfap(B1_OFF, HK))
                    lt2 = mp.tile([P, 3 * H], F32)
                    nc.sync.dma_start(out=lt2[:], in_=wfap(B2_OFF, 3 * H))
                    b2_sb = lt2[:, 0:H]
                    l2g_sb = lt2[:, H:2 * H]
                    l2b_sb = lt2[:, 2 * H:3 * H]
                    # w1: blocks 0-3 were preloaded persistently after proj;
                    # blocks 4-5 stream here, overlapped with the first
                    # chunk's matmuls on blocks 0-3
                    w1sb = w1pre + [
                        mp.tile([P, CK, P], BF16, name=f"w1sb{f}")
                        for f in range(npre, HK)
                    ]
                    for n in range(5):  # 512-wide node chunks
                        ns = slice(n * 512, (n + 1) * 512)
                        zt = [
                            mp.tile([P, 512], BF16, tag=f"zt{f}", bufs=2, name=f"zt{f}")
                            for f in range(HK)
                        ]
                        for f in range(HK):
                            if n == 0 and f >= npre:
                                nc.sync.dma_start(
                                    out=w1sb[f][:],
                                    in_=wb[0:1, W1_OFF + f * P * CK * P:
                                           W1_OFF + (f + 1) * P * CK * P].rearrange(
                                        "o (p k n) -> (o p) k n", p=P, n=P
                                    ),
                                )
                            pz = psum.tile([P, 512], F32, tag="z", bufs=2)
                            for kk in range(CK):
                                nc.tensor.matmul(
                                    out=pz[:],
                                    lhsT=w1sb[f][:, kk, :],
                                    rhs=ct[kk][:, ns],
                                    start=(kk == 0),
                                    stop=(kk == CK - 1),
                                )
                            nc.scalar.activation(
                                out=zt[f][:],
                                in_=pz[:],
                                func=mybir.ActivationFunctionType.Gelu,
                                bias=b1_sb[:, f:f + 1],
                            )
                        for mm in range(4):
                            m = n * 4 + mm
                            po = psum.tile([P, H], F32, tag="o", bufs=2)
                            for n0, nn in NSL:
                                for f in range(HK):
                                    nc.tensor.matmul(
                                        out=po[:, n0:n0 + nn],
                                        lhsT=zt[f][:, mm * P:(mm + 1) * P],
                                        rhs=w2t[f][:, n0:n0 + nn],
                                        start=(f == 0),
                                        stop=(f == HK - 1),
                                    )
                            t0 = mp.tile([P, H], F32, tag="t0", bufs=2)
                            nc.vector.tensor_add(out=t0[:], in0=po[:], in1=b2_sb)
                            mu = mp.tile([P, 1], F32, tag="mu", bufs=2)
                            nc.vector.reduce_sum(
                                out=mu[:], in_=t0[:], axis=mybir.AxisListType.X
                            )
                            sq = mp.tile([P, H], F32, tag="sq", bufs=2)
                            nc.scalar.activation(
                                out=sq[:], in_=t0[:],
                                func=mybir.ActivationFunctionType.Square,
                            )
                            var = mp.tile([P, 1], F32, tag="var", bufs=2)
                            nc.vector.reduce_sum(
                                out=var[:], in_=sq[:], axis=mybir.AxisListType.X
                            )
                            nc.scalar.mul(out=mu[:], in_=mu[:], mul=1.0 / H)
                            m2 = mp.tile([P, 1], F32, tag="m2", bufs=2)
                            nc.vector.tensor_mul(out=m2[:], in0=mu[:], in1=mu[:])
                            nc.vector.tensor_scalar(
                                out=var[:], in0=var[:], scalar1=1.0 / H,
                                scalar2=None, op0=mybir.AluOpType.mult,
                            )
                            nc.vector.tensor_sub(out=var[:], in0=var[:], in1=m2[:])
                            rs = mp.tile([P, 1], F32, tag="rs", bufs=2)
                            nc.scalar.activation(
                                out=rs[:], in_=var[:],
                                func=mybir.ActivationFunctionType.Sqrt,
                                bias=eps_t[:, :1],
                            )
                            nc.vector.reciprocal(out=rs[:], in_=rs[:])
                            nc.vector.tensor_scalar(
                                out=t0[:], in0=t0[:], scalar1=mu[:, :1],
                                scalar2=rs[:, :1],
                                op0=mybir.AluOpType.subtract,
                                op1=mybir.AluOpType.mult,
                            )
                            nc.vector.tensor_mul(out=t0[:], in0=t0[:], in1=l2g_sb)
                            ot = mp.tile([P, H], F32, tag="ot", bufs=2)
                            nc.vector.tensor_add(out=ot[:], in0=t0[:], in1=l2b_sb)
                            nc.sync.dma_start(out=out[m * P:(m + 1) * P, :], in_=ot[:])
    nc.compile()
    return nc


def check_waits(nc, limit=1):
    """Return list of DMA-queue instructions exceeding the wait limit."""
    bad = []
    for f in nc.m.functions:
        for bb in f.blocks:
            for ins in bb.instructions:
                tn = type(ins).__name__
                if tn not in ("InstDMACopy", "InstDmaTransposeAnt"):
                    continue
                si = ins.sync_info
                if len(si.on_wait) > limit:
                    bad.append(
                        (ins.name, tn, str(ins.engine),
                         [(w.ant_name, w.wait_value) for w in si.on_wait])
                    )
    return bad


# ------------------------------------------------------------- entry point

def _in_maps(inputs, nch, gidx, oh, idx16):
    x = np.asarray(inputs["x"], dtype=np.float32)
    bcast = lambda v: np.broadcast_to(
        np.asarray(v, np.float32), (P, H)
    ).copy()
    gb = np.asarray(inputs["gcn_b"], np.float32)  # [HOPS, H]
    gbcol = np.zeros((P, HOPS * HK), np.float32)
    for k in range(HOPS):
        for f in range(HK):
            gbcol[:, k * HK + f] = gb[k, f * P:(f + 1) * P]
    b1 = np.asarray(inputs["mlp_b1"], np.float32)
    b1col = np.zeros((P, HK), np.float32)
    for f in range(HK):
        b1col[:, f] = b1[f * P:(f + 1) * P]
    w1 = np.asarray(inputs["mlp_w1"], np.float32)  # [3072, 768]
    w1p = np.zeros((P, HK * (HOPS + 1) * H), np.float32)
    for f in range(HK):
        blk = w1[:, f * P:(f + 1) * P]  # [3072, 128]
        w1p[:, f * CK * P:(f + 1) * CK * P] = (
            blk.reshape(CK, P, P).transpose(1, 0, 2).reshape(P, CK * P)
        )
    w1flat = np.concatenate(
        [w1p[:, f * CK * P:(f + 1) * CK * P].ravel() for f in range(HK)]
    )
    wb = np.concatenate([
        np.asarray(inputs["proj_w"], np.float32).ravel(),
        np.asarray(inputs["gcn_w"], np.float32).ravel(),
        np.asarray(inputs["mlp_w2"], np.float32).ravel(),
        w1flat,
    ]).astype(BF).reshape(1, WBTOT)
    wf = np.concatenate([
        bcast(inputs["proj_b"]),
        bcast(inputs["ln1_g"]),
        bcast(inputs["ln1_b"]),
        bcast(inputs["mlp_b2"]),
        bcast(inputs["ln2_g"]),
        bcast(inputs["ln2_b"]),
        gbcol,
        b1col,
    ], axis=1).astype(np.float32)
    assert wf.shape == (P, FTOT)
    common = {"wb": wb, "wf": wf}
    maps = []
    for c in range(NCORES):
        xc = np.zeros((NPAD, IN_F), np.float32)
        xc[:NLOC] = x[c * NLOC:(c + 1) * NLOC]
        # pack: xp[p, m*IN_F + k*128 + n] = x[m*128+n, k*128+p]
        xp = (
            xc.reshape(MT, P, INK, P)      # [m, n, k, p]
            .transpose(3, 0, 2, 1)          # [p, m, k, n]
            .reshape(P, MT * IN_F)
        )
        ohp = (
            oh[c].astype(np.float32).reshape(-1, P, P)  # [ch, p, d]
            .transpose(1, 0, 2)                          # [p, ch, d]
            .reshape(P, -1)
        )
        m = dict(common)
        m["xT"] = xp.astype(BF)
        m["gidx"] = idx16[c]
        m["oh"] = ohp.astype(BF)
        maps.append(m)
    return maps


def kernel(**inputs):
    nch, c_list, gidx, oh, idx16, nech = _prep(np.asarray(inputs["edge_index"]))
    nc = _build(nch, c_list, nech, stage=4)
    maps = _in_maps(inputs, nch, gidx, oh, idx16)
    res = run_bass_kernel_spmd(nc, maps, list(range(NCORES)))
    outs = [res.results[c]["out"][:NLOC] for c in range(NCORES)]
    return np.concatenate(outs, axis=0).astype(np.float32)



# revision 25
# speedup vs baseline: 1.3484x; 1.1860x over previous
"""GCN context-paper kernel for 8 trn2 NeuronCores (SPMD via bass/Tile).

Model (see reference): proj+LN -> 3x GCNConv(+self loops, sym-norm) with
GELU -> concat(4 hops) -> MLP(GELU) -> LN.

Sharding: nodes partitioned across 8 cores (2500/core, padded to 2560).
Per hop: each core computes Y = h @ W for its nodes, AllGathers Y (bf16),
then builds its nodes' aggregation with indirect row-gathers of Y plus
one-hot matmuls on the tensor engine (edge weights folded into the
one-hot values); self-loop terms read local Y straight from SBUF.

Layout strategy: activations are kept feature-major ("ct" tiles,
[128 feat, 2560 nodes]) which the scatter matmul produces directly and
all lhsT uses consume directly; only the proj output needs PE transposes.

Perf notes (measured on the axon-tunneled 8-core setup):
- Per-exec dispatch cost through the PJRT tunnel is ~40 us PER OPERAND,
  so all inputs are consolidated into two tensors: per-core `xc`
  (x^T | one-hot | gather idx | f32 tables, bitcast into bf16 columns)
  and shared flat weights `wb`.
- Each collective has a ~120 us fixed cost here, so ONE full-width
  AllGather per hop beats a feature-halved pair (split_ag=False);
  per-tile agin streaming lets it start right after the last Y matmul.
- LN uses var = E[x^2]-mu^2 (square on ACT overlaps mean-reduce on DVE)
  and a fused (x-mu)*rs tensor_scalar.

DMA discipline: every DMA-queue instruction must end up with at most ONE
semaphore wait (hardware struct limit). Hence: DMA destinations in SBUF
are either fresh tiles or have engine-op (not DMA) prior writers; DMA
sources are external inputs or covered by dummy lane-warming DMAs
(collective output).
"""

import numpy as np
import ml_dtypes

import concourse.bass as bass
import concourse.bacc as bacc
import concourse.mybir as mybir
import concourse.tile as tile
from concourse.bass_utils import run_bass_kernel_spmd
from concourse.masks import make_identity

# problem constants (hardcoded per contract)
N, E, IN_F, H, HOPS = 20000, 100000, 1536, 768, 3
LN_EPS = 1e-5
NCORES = 8
NLOC = N // NCORES            # 2500 real nodes per core
P = 128
MT = 20                       # node tiles per core
NPAD = MT * P                 # 2560 padded nodes per core
HK = H // P                   # 6 feature tiles
HH = H // 2                   # feature half width (AllGather split)
HKH = HK // 2                 # feature tiles per half
INK = IN_F // P               # 12
CK = (HOPS + 1) * H // P      # 24 cat feature tiles
NSL = ((0, 512), (512, 256))  # N-dim slices for 768-wide outputs
OB = 8                        # chunks per one-hot load
GRP = 8                       # edge chunks per dma_gather
NWARM = 4                     # dummy lane-warming DMAs per collective

# flat offsets into the consolidated bf16 weight tensor (elements)
PROJ_OFF = 0
GCN_OFF = PROJ_OFF + IN_F * H
W2_OFF = GCN_OFF + HOPS * H * H
W1_OFF = W2_OFF + H * H
WBTOT = W1_OFF + (HOPS + 1) * H * H
# flat offsets into the consolidated f32 table section (per-partition cols)
PB_OFF, L1G_OFF, L1B_OFF = 0, H, 2 * H
B2_OFF, L2G_OFF, L2B_OFF = 3 * H, 4 * H, 5 * H
GB_OFF = 6 * H
B1_OFF = GB_OFF + HOPS * HK
FTOT = B1_OFF + HK


def _pad16(x):
    return (x + 15) // 16 * 16


def _xc_layout(nch, nech):
    """bf16-column offsets of the sections in the per-core xc tensor."""
    xt_off = 0
    oh_off = _pad16(xt_off + MT * IN_F)
    idx_off = _pad16(oh_off + nch * P)
    wf_off = _pad16(idx_off + (nech * P) // 16)
    tot = _pad16(wf_off + 2 * FTOT)
    return xt_off, oh_off, idx_off, wf_off, tot

F32 = mybir.dt.float32
BF16 = mybir.dt.bfloat16
I32 = mybir.dt.int32
BF = ml_dtypes.bfloat16


# ---------------------------------------------------------------- host prep

def _prep(edge_index):
    """Host preprocessing: normalization, edge sorting, per-core chunk
    tables (gather indices + one-hot weight blocks)."""
    src = np.asarray(edge_index[0], dtype=np.int64)
    dst = np.asarray(edge_index[1], dtype=np.int64)
    deg = np.bincount(dst, minlength=N).astype(np.float64) + 1.0
    dis = 1.0 / np.sqrt(deg)

    # real edges only; self loops become a dedicated per-tile identity chunk
    alls, alld = src, dst
    w = (dis[alls] * dis[alld]).astype(np.float32)

    # global row in the AllGather output for each source node
    yg_row = (alls // NLOC) * NPAD + (alls % NLOC)

    # group edges by (core, dst tile)
    core = alld // NLOC
    loc = alld % NLOC
    t = loc // P
    d = loc % P  # local offset within dst tile
    counts = np.zeros((NCORES, MT), dtype=np.int64)
    np.add.at(counts, (core, t), 1)
    # chunk 0 of each tile = self loops (plain DMA from local Y); rest edges
    c_list = [1 + max(1, int(np.ceil(counts[:, tt].max() / P))) for tt in range(MT)]
    off = np.zeros(MT, dtype=np.int64)
    off[1:] = np.cumsum(c_list)[:-1]
    nch = int(sum(c_list))

    gidx = np.zeros((NCORES, P, nch), dtype=np.int32)
    oh = np.zeros((NCORES, nch * P, P), dtype=np.float32)

    # self chunks: diag(dis[d]^2) per (core, tile)
    for cc in range(NCORES):
        for tt in range(MT):
            nreal = min(P, NLOC - tt * P)
            gl = cc * NLOC + tt * P + np.arange(nreal)
            ch = off[tt]
            oh[cc, ch * P + np.arange(nreal), np.arange(nreal)] = (
                dis[gl] * dis[gl]
            )

    order = np.lexsort((alls, t, core))  # stable ordering by (core, tile)
    so_core, so_t, so_d = core[order], t[order], d[order]
    so_w, so_yg = w[order], yg_row[order]
    grp = so_core * MT + so_t
    start = np.zeros(NCORES * MT + 1, dtype=np.int64)
    np.add.at(start, grp + 1, 1)
    start = np.cumsum(start)
    pos = np.arange(len(order)) - start[grp]
    chunk = off[so_t] + 1 + pos // P
    row = pos % P
    gidx[so_core, row, chunk] = so_yg.astype(np.int32)
    oh[so_core, chunk * P + row, so_d] = so_w

    # int16 index stream for dma_gather: edge-chunk ids exclude self chunks
    n_self_before = np.zeros(nch, dtype=np.int64)
    for tt in range(MT):
        n_self_before[off[tt]:] += 0  # placeholder
    # chunk -> edge-chunk id: subtract #self chunks with index <= chunk
    selfmask = np.zeros(nch, dtype=np.int64)
    selfmask[off] = 1
    ech_of = np.cumsum(selfmask) - 1  # for self chunks: id of tile
    ech_map = np.arange(nch) - np.cumsum(selfmask)  # edge-chunk id (c>0)
    nech = nch - MT
    ni_tot = nech * P
    idx16 = np.zeros((NCORES, 128, ni_tot // 16), dtype=np.int16)
    e_ch = ech_map[chunk]  # edge-chunk id per sorted edge
    i_flat = e_ch * P + row
    p16 = i_flat % 16
    c16 = i_flat // 16
    for cc in range(NCORES):
        m = so_core == cc
        a = np.zeros((16, ni_tot // 16), np.int16)
        a[p16[m], c16[m]] = so_yg[m].astype(np.int16)
        idx16[cc] = np.tile(a, (8, 1))
    return nch, c_list, gidx, oh.astype(BF), idx16, nech


# --------------------------------------------------------------- bass build

def _build(nch, c_list, nech, stage=4, fake_ag=False, split_ag=False):
    """Emit the SPMD Bass program. stage: 1=proj only, 2=+1 hop,
    3=+3 hops, 4=full (MLP+LN2). For stage<4 the output is the ct
    (feature-major) tiles of the last computed hop, [768, NPAD] f32."""
    nc = bacc.Bacc(
        "TRN2", target_bir_lowering=False, debug=False, num_devices=NCORES,
        num_swdge_queues=4,
    )
    dp = nc.declare_dram_parameter
    # consolidated operands: fewer kernel params -> much lower per-exec
    # dispatch cost through the PJRT tunnel (~40 us/operand measured).
    # xc = [x^T | one-hot tables | gather indices (bitcast) | f32 tables
    # (bitcast)], all per-core; wb = shared flat bf16 weights.
    XT_OFF, OH_OFF, IDX_OFF, WF_OFF, XCTOT = _xc_layout(nch, nech)
    xc = dp("xc", [P, XCTOT], BF16, isOutput=False)
    wb = dp("wb", [1, WBTOT], BF16, isOutput=False)

    def wb2d(off, rows, cols):
        # contiguous [rows, cols] block at flat offset `off` in wb
        return wb[0:1, off:off + rows * cols].rearrange(
            "o (p h) -> (o p) h", p=rows
        )

    def wfap(off, width):
        # f32 view into the per-core xc tensor
        return xc[:, WF_OFF + 2 * off:WF_OFF + 2 * (off + width)].bitcast(F32)

    nhop = 0 if stage <= 1 else (1 if stage == 2 else HOPS)
    if stage >= 4:
        out = dp("out", [NPAD, H], F32, isOutput=True)
    else:
        out = dp("out", [H, NPAD], F32, isOutput=True)

    off = np.zeros(MT, dtype=np.int64)
    off[1:] = np.cumsum(c_list)[:-1]

    with tile.TileContext(nc) as tc:
        import contextlib

        with contextlib.ExitStack() as ctx:
            dram = ctx.enter_context(tc.tile_pool(name="dram", bufs=1, space="DRAM"))
            cat = ctx.enter_context(tc.tile_pool(name="cat", bufs=1))
            cst = ctx.enter_context(tc.tile_pool(name="cst", bufs=1))

            # persistent feature-major activation tiles
            ct = [cat.tile([P, NPAD], BF16, name=f"ct{i}") for i in range(CK)]
            # persistent slabs for 4 of the 6 w1 blocks (loaded after proj
            # so the MLP doesn't stall on its weight stream); only fits
            # when the halved gather tiles free enough SBUF
            npre = 4 if split_ag else 0
            w1pre = [cat.tile([P, CK, P], BF16, name=f"w1pre{f}")
                     for f in range(npre)]

            idx_sb = cst.tile([128, (nech * P) // 16], mybir.dt.int16)
            gb_sb = cst.tile([P, HOPS * HK], F32)
            ident = cst.tile([P, P], BF16)
            make_identity(nc, ident[:])
            eps_t = cst.tile([P, 1], F32)
            nc.gpsimd.memset(eps_t[:], LN_EPS)

            # ---------------- proj + LN1 -> ct[0..5] (via PE transpose)
            with tc.tile_pool(name="proj", bufs=1) as pp, \
                    tc.tile_pool(name="psum_pj", bufs=1, space="PSUM") as psum:
                # first x slab first so the PE can start ASAP
                xs0 = pp.tile([P, INK, P], BF16, tag="xslab", bufs=3)
                nc.sync.dma_start(
                    out=xs0[:],
                    in_=xc[:, XT_OFF:XT_OFF + IN_F].rearrange(
                        "p (k n) -> p k n", n=P
                    ),
                )
                pw = [pp.tile([P, H], BF16, name=f"pw{k}") for k in range(INK)]
                for k in range(INK):
                    nc.sync.dma_start(
                        out=pw[k][:], in_=wb2d(PROJ_OFF + k * P * H, P, H)
                    )
                lt1 = pp.tile([P, 3 * H], F32)
                nc.sync.dma_start(out=lt1[:], in_=wfap(PB_OFF, 3 * H))
                pb_sb = lt1[:, 0:H]
                l1g_sb = lt1[:, H:2 * H]
                l1b_sb = lt1[:, 2 * H:3 * H]

                for m in range(MT):
                    ms = slice(m * P, (m + 1) * P)
                    if m == 0:
                        xs = xs0
                    else:
                        xs = pp.tile([P, INK, P], BF16, tag="xslab", bufs=3)
                        nc.sync.dma_start(
                            out=xs[:],
                            in_=xc[:, XT_OFF + m * IN_F:
                                   XT_OFF + (m + 1) * IN_F].rearrange(
                                "p (k n) -> p k n", n=P
                            ),
                        )
                    ps = psum.tile([P, H], F32, tag="pj", bufs=2)
                    for n0, nn in NSL:
                        for k in range(INK):
                            nc.tensor.matmul(
                                out=ps[:, n0:n0 + nn],
                                lhsT=xs[:, k, :],
                                rhs=pw[k][:, n0:n0 + nn],
                                start=(k == 0),
                                stop=(k == INK - 1),
                            )
                    # LN1 over features (free dim), node-major.
                    # var = E[x^2] - mu^2 so the square (on ACT) overlaps the
                    # mean reduce (on DVE); Rsqrt + fused (x-mu)*rs cut DVE ops.
                    t0 = pp.tile([P, H], F32, tag="t0", bufs=2)
                    nc.vector.tensor_add(out=t0[:], in0=ps[:], in1=pb_sb)
                    mu = pp.tile([P, 1], F32, tag="mu", bufs=2)
                    nc.vector.reduce_sum(out=mu[:], in_=t0[:], axis=mybir.AxisListType.X)
                    sq = pp.tile([P, H], F32, tag="sq", bufs=2)
                    nc.scalar.activation(
                        out=sq[:], in_=t0[:],
                        func=mybir.ActivationFunctionType.Square,
                    )
                    var = pp.tile([P, 1], F32, tag="var", bufs=2)
                    nc.vector.reduce_sum(out=var[:], in_=sq[:], axis=mybir.AxisListType.X)
                    nc.scalar.mul(out=mu[:], in_=mu[:], mul=1.0 / H)
                    m2 = pp.tile([P, 1], F32, tag="m2", bufs=2)
                    nc.vector.tensor_mul(out=m2[:], in0=mu[:], in1=mu[:])
                    nc.vector.tensor_scalar(
                        out=var[:], in0=var[:], scalar1=1.0 / H, scalar2=None,
                        op0=mybir.AluOpType.mult,
                    )
                    nc.vector.tensor_sub(out=var[:], in0=var[:], in1=m2[:])
                    rs = pp.tile([P, 1], F32, tag="rs", bufs=2)
                    nc.scalar.activation(
                        out=rs[:], in_=var[:],
                        func=mybir.ActivationFunctionType.Sqrt,
                        bias=eps_t[:, :1],
                    )
                    nc.vector.reciprocal(out=rs[:], in_=rs[:])
                    nc.vector.tensor_scalar(
                        out=t0[:], in0=t0[:], scalar1=mu[:, :1], scalar2=rs[:, :1],
                        op0=mybir.AluOpType.subtract, op1=mybir.AluOpType.mult,
                    )
                    nc.vector.tensor_mul(out=t0[:], in0=t0[:], in1=l1g_sb)
                    h0 = pp.tile([P, H], BF16, tag="h0", bufs=2)
                    nc.vector.tensor_add(out=h0[:], in0=t0[:], in1=l1b_sb)
                    # transpose 6 blocks -> ct[f][:, m]
                    for f in range(HK):
                        tp = psum.tile([P, P], BF16, tag="tp", bufs=2)
                        nc.tensor.transpose(
                            out=tp[:], in_=h0[:, f * P:(f + 1) * P], identity=ident[:]
                        )
                        nc.vector.tensor_copy(out=ct[f][:, ms], in_=tp[:])

            # deferred loads: not needed until the first scatter pass / MLP,
            # so keep them off the critical startup DMA queue
            nc.sync.dma_start(
                out=idx_sb[:],
                in_=xc[:, IDX_OFF:IDX_OFF + (nech * P) // 16].bitcast(
                    mybir.dt.int16
                ),
            )
            nc.sync.dma_start(out=gb_sb[:], in_=wfap(GB_OFF, HOPS * HK))
            for f in range(npre):
                nc.sync.dma_start(
                    out=w1pre[f][:],
                    in_=wb[0:1, W1_OFF + f * P * CK * P:
                           W1_OFF + (f + 1) * P * CK * P].rearrange(
                        "o (p k n) -> (o p) k n", p=P, n=P
                    ),
                )

            # ---------------- hops
            for k in range(nhop):
                hp = tc.tile_pool(name=f"hop{k}", bufs=1)
                with hp as hpool, \
                        tc.tile_pool(name=f"psum_h{k}", bufs=1, space="PSUM") as psum:
                    gw = [hpool.tile([P, H], BF16, name=f"gw{k}_{f}") for f in range(HK)]
                    for f in range(HK):
                        nc.sync.dma_start(
                            out=gw[f][:],
                            in_=wb2d(GCN_OFF + (k * H + f * P) * H, P, H),
                        )
                    ybig = hpool.tile([P, MT * H], BF16)
                    # feature-halved AllGather: gather/scatter of half 0
                    # overlaps the collective for half 1 (split_ag=True);
                    # otherwise one full-width AllGather
                    nhalf = 2 if split_ag else 1
                    hw_ = HH if split_ag else H
                    agins = [
                        dram.tile([NPAD, hw_], BF16, name=f"agin{k}_{h}")
                        for h in range(nhalf)
                    ]
                    ygs = [
                        dram.tile([NCORES * NPAD, hw_], BF16, addr_space="Shared",
                                  name=f"yg{k}_{h}")
                        for h in range(nhalf)
                    ]
                    for m in range(MT):
                        ms = slice(m * P, (m + 1) * P)
                        ps = psum.tile([P, H], F32, tag="y", bufs=2)
                        for n0, nn in NSL:
                            for f in range(HK):
                                nc.tensor.matmul(
                                    out=ps[:, n0:n0 + nn],
                                    lhsT=ct[6 * k + f][:, ms],
                                    rhs=gw[f][:, n0:n0 + nn],
                                    start=(f == 0),
                                    stop=(f == HK - 1),
                                )
                        nc.vector.tensor_copy(out=ybig[:, m * H:(m + 1) * H], in_=ps[:])
                        # stream this tile's rows to DRAM immediately so the
                        # AllGather can start right after the last Y matmul
                        nc.sync.dma_start(
                            out=agins[0][m * P:(m + 1) * P, :],
                            in_=ybig[:, m * H:m * H + hw_],
                        )
                        if split_ag:
                            nc.scalar.dma_start(
                                out=agins[1][m * P:(m + 1) * P, :],
                                in_=ybig[:, m * H + HH:(m + 1) * H],
                            )
                    for h in range(nhalf):
                        if fake_ag:
                            # timing-proxy only: local copy standing in for the
                            # AllGather (the sim's collective model is ~12x
                            # pessimistic for intra-chip groups)
                            nc.gpsimd.dma_start(out=ygs[h][0:NPAD, :], in_=agins[h][:])
                        else:
                            nc.gpsimd.collective_compute(
                                "AllGather",
                                mybir.AluOpType.bypass,
                                ins=[agins[h].opt()],
                                outs=[ygs[h].opt()],
                                replica_groups=[list(range(NCORES))],
                            )

                    # scatter passes, one per feature half
                    nch_tot = int(sum(c_list))
                    for half in range(nhalf):
                        yg = ygs[half]
                        # warm SWDGE lanes with 1-dep dummy reads of yg
                        for dlane in range(NWARM):
                            dmy = hpool.tile([2, 4], BF16,
                                             tag=f"dmy{half}_{dlane}", bufs=1)
                            nc.gpsimd.dma_start(
                                out=dmy[:], in_=yg[dlane * 2:dlane * 2 + 2, 0:4]
                            )
                        if k == 0 and half == 0:
                            # warm lanes on the idx region too (SBUF->SBUF tiny)
                            for dlane in range(NWARM):
                                dmi = hpool.tile([2, 1], I32, tag=f"dmi{dlane}",
                                                 bufs=1)
                                nc.gpsimd.dma_start(
                                    out=dmi[:], in_=idx_sb[dlane:dlane + 2, 0:1]
                                )
                        oh_tiles = {}
                        g_tiles = {}
                        ech = 0  # running edge-chunk id
                        for t in range(MT):
                            ts = slice(t * P, (t + 1) * P)
                            npa = HKH if split_ag else HK
                            if split_ag:
                                pa = psum.tile([P, HKH * P], F32,
                                               tag=f"sc{half}", bufs=2)
                                pb_ = None
                            else:
                                pa = psum.tile([P, 512], F32, tag="sca", bufs=2)
                                pb_ = psum.tile([P, 256], F32, tag="scb", bufs=2)

                            def _dst(fi):
                                if pb_ is None or fi < 4:
                                    return pa[:, (fi % 4) * P:(fi % 4 + 1) * P]
                                return pb_[:, (fi - 4) * P:(fi - 3) * P]

                            starts = (0,) if pb_ is None else (0, 4)
                            stops = (npa - 1,) if pb_ is None else (3, 5)
                            for c in range(c_list[t]):
                                ch = int(off[t]) + c
                                if c == 0:
                                    # self-loop chunk: local Y rows already in
                                    # SBUF node-major (ybig)
                                    gsl = ybig[:, t * H + half * HH:
                                               t * H + half * HH + hw_]
                                else:
                                    gg, gj = ech // GRP, ech % GRP
                                    if gj == 0:
                                        ng = min(GRP, nech - gg * GRP)
                                        gt = hpool.tile(
                                            [P, ng, hw_], BF16, tag="g",
                                            bufs=2, name=f"g{k}_{half}_{gg}",
                                        )
                                        nc.gpsimd.dma_gather(
                                            out_ap=gt[:],
                                            in_ap=yg[:],
                                            idxs_ap=idx_sb[
                                                :, gg * GRP * 8:(gg * GRP + ng) * 8
                                            ],
                                            num_idxs=ng * P,
                                            num_idxs_reg=ng * P,
                                            elem_size=hw_,
                                            queue_num=gg % 4,
                                        )
                                        g_tiles[gg] = gt
                                    gsl = g_tiles[gg][:, gj, :]
                                    ech += 1
                                og, oj = ch // OB, ch % OB
                                if oj == 0:
                                    no = min(OB, nch_tot - og * OB)
                                    oh_t = hpool.tile(
                                        [P, no, P], BF16, tag="oh", bufs=3,
                                        name=f"oh{k}_{half}_{og}",
                                    )
                                    nc.sync.dma_start(
                                        out=oh_t[:],
                                        in_=xc[
                                            :, OH_OFF + og * OB * P:
                                            OH_OFF + (og * OB + no) * P
                                        ].rearrange("p (c m) -> p c m", m=P),
                                    )
                                    oh_tiles[og] = oh_t
                                oh_t = oh_tiles[og]
                                first, last = (c == 0), (c == c_list[t] - 1)
                                for fi in range(npa):
                                    nc.tensor.matmul(
                                        out=_dst(fi),
                                        lhsT=gsl[:, fi * P:(fi + 1) * P],
                                        rhs=oh_t[:, oj, :],
                                        start=first and fi in starts,
                                        stop=last and fi in stops,
                                    )
                            for fi in range(npa):
                                f = half * HKH + fi
                                nc.scalar.activation(
                                    out=ct[6 * (k + 1) + f][:, ts],
                                    in_=_dst(fi),
                                    func=mybir.ActivationFunctionType.Gelu,
                                    bias=gb_sb[:, k * HK + f:k * HK + f + 1],
                                )

            if stage < 4:
                # dump last hop's ct tiles as [H, NPAD] f32
                with tc.tile_pool(name="dump", bufs=1) as dpool:
                    for f in range(HK):
                        df = dpool.tile([P, NPAD], F32, tag="df", bufs=2)
                        nc.vector.tensor_copy(out=df[:], in_=ct[6 * nhop + f][:])
                        nc.sync.dma_start(out=out[f * P:(f + 1) * P, :], in_=df[:])

            if stage >= 4:
                # ---------------- MLP + LN2
                with tc.tile_pool(name="mlp", bufs=1) as mp, \
                        tc.tile_pool(name="psum_mlp", bufs=1, space="PSUM") as psum:
                    w2t = [mp.tile([P, H], BF16, name=f"w2t{f}") for f in range(HK)]
                    for f in range(HK):
                        nc.sync.dma_start(
                            out=w2t[f][:], in_=wb2d(W2_OFF + f * P * H, P, H)
                        )
                    b1_sb = mp.tile([P, HK], F32)
                    nc.sync.dma_start(out=b1_sb[:], in_=wfap(B1_OFF, HK))
                    lt2 = mp.tile([P, 3 * H], F32)
                    nc.sync.dma_start(out=lt2[:], in_=wfap(B2_OFF, 3 * H))
                    b2_sb = lt2[:, 0:H]
                    l2g_sb = lt2[:, H:2 * H]
                    l2b_sb = lt2[:, 2 * H:3 * H]
                    # w1: blocks 0-3 were preloaded persistently after proj;
                    # blocks 4-5 stream here, overlapped with the first
                    # chunk's matmuls on blocks 0-3
                    w1sb = w1pre + [
                        mp.tile([P, CK, P], BF16, name=f"w1sb{f}")
                        for f in range(npre, HK)
                    ]
                    for n in range(5):  # 512-wide node chunks
                        ns = slice(n * 512, (n + 1) * 512)
                        zt = [
                            mp.tile([P, 512], BF16, tag=f"zt{f}", bufs=2, name=f"zt{f}")
                            for f in range(HK)
                        ]
                        for f in range(HK):
                            if n == 0 and f >= npre:
                                nc.sync.dma_start(
                                    out=w1sb[f][:],
                                    in_=wb[0:1, W1_OFF + f * P * CK * P:
                                           W1_OFF + (f + 1) * P * CK * P].rearrange(
                                        "o (p k n) -> (o p) k n", p=P, n=P
                                    ),
                                )
                            pz = psum.tile([P, 512], F32, tag="z", bufs=2)
                            for kk in range(CK):
                                nc.tensor.matmul(
                                    out=pz[:],
                                    lhsT=w1sb[f][:, kk, :],
                                    rhs=ct[kk][:, ns],
                                    start=(kk == 0),
                                    stop=(kk == CK - 1),
                                )
                            nc.scalar.activation(
                                out=zt[f][:],
                                in_=pz[:],
                                func=mybir.ActivationFunctionType.Gelu,
                                bias=b1_sb[:, f:f + 1],
                            )
                        for mm in range(4):
                            m = n * 4 + mm
                            po = psum.tile([P, H], F32, tag="o", bufs=2)
                            for n0, nn in NSL:
                                for f in range(HK):
                                    nc.tensor.matmul(
                                        out=po[:, n0:n0 + nn],
                                        lhsT=zt[f][:, mm * P:(mm + 1) * P],
                                        rhs=w2t[f][:, n0:n0 + nn],
                                        start=(f == 0),
                                        stop=(f == HK - 1),
                                    )
                            t0 = mp.tile([P, H], F32, tag="t0", bufs=2)
                            nc.vector.tensor_add(out=t0[:], in0=po[:], in1=b2_sb)
                            mu = mp.tile([P, 1], F32, tag="mu", bufs=2)
                            nc.vector.reduce_sum(
                                out=mu[:], in_=t0[:], axis=mybir.AxisListType.X
                            )
                            sq = mp.tile([P, H], F32, tag="sq", bufs=2)
                            nc.scalar.activation(
                                out=sq[:], in_=t0[:],
                                func=mybir.ActivationFunctionType.Square,
                            )
                            var = mp.tile([P, 1], F32, tag="var", bufs=2)
                            nc.vector.reduce_sum(
                                out=var[:], in_=sq[:], axis=mybir.AxisListType.X
                            )
                            nc.scalar.mul(out=mu[:], in_=mu[:], mul=1.0 / H)
                            m2 = mp.tile([P, 1], F32, tag="m2", bufs=2)
                            nc.vector.tensor_mul(out=m2[:], in0=mu[:], in1=mu[:])
                            nc.vector.tensor_scalar(
                                out=var[:], in0=var[:], scalar1=1.0 / H,
                                scalar2=None, op0=mybir.AluOpType.mult,
                            )
                            nc.vector.tensor_sub(out=var[:], in0=var[:], in1=m2[:])
                            rs = mp.tile([P, 1], F32, tag="rs", bufs=2)
                            nc.scalar.activation(
                                out=rs[:], in_=var[:],
                                func=mybir.ActivationFunctionType.Sqrt,
                                bias=eps_t[:, :1],
                            )
                            nc.vector.reciprocal(out=rs[:], in_=rs[:])
                            nc.vector.tensor_scalar(
                                out=t0[:], in0=t0[:], scalar1=mu[:, :1],
                                scalar2=rs[:, :1],
                                op0=mybir.AluOpType.subtract,
                                op1=mybir.AluOpType.mult,
                            )
                            nc.vector.tensor_mul(out=t0[:], in0=t0[:], in1=l2g_sb)
                            ot = mp.tile([P, H], F32, tag="ot", bufs=2)
                            nc.vector.tensor_add(out=ot[:], in0=t0[:], in1=l2b_sb)
                            nc.sync.dma_start(out=out[m * P:(m + 1) * P, :], in_=ot[:])
    nc.compile()
    return nc


def check_waits(nc, limit=1):
    """Return list of DMA-queue instructions exceeding the wait limit."""
    bad = []
    for f in nc.m.functions:
        for bb in f.blocks:
            for ins in bb.instructions:
                tn = type(ins).__name__
                if tn not in ("InstDMACopy", "InstDmaTransposeAnt"):
                    continue
                si = ins.sync_info
                if len(si.on_wait) > limit:
                    bad.append(
                        (ins.name, tn, str(ins.engine),
                         [(w.ant_name, w.wait_value) for w in si.on_wait])
                    )
    return bad


# ------------------------------------------------------------- entry point

def _in_maps(inputs, nch, gidx, oh, idx16):
    x = np.asarray(inputs["x"], dtype=np.float32)
    bcast = lambda v: np.broadcast_to(
        np.asarray(v, np.float32), (P, H)
    ).copy()
    gb = np.asarray(inputs["gcn_b"], np.float32)  # [HOPS, H]
    gbcol = np.zeros((P, HOPS * HK), np.float32)
    for k in range(HOPS):
        for f in range(HK):
            gbcol[:, k * HK + f] = gb[k, f * P:(f + 1) * P]
    b1 = np.asarray(inputs["mlp_b1"], np.float32)
    b1col = np.zeros((P, HK), np.float32)
    for f in range(HK):
        b1col[:, f] = b1[f * P:(f + 1) * P]
    w1 = np.asarray(inputs["mlp_w1"], np.float32)  # [3072, 768]
    w1p = np.zeros((P, HK * (HOPS + 1) * H), np.float32)
    for f in range(HK):
        blk = w1[:, f * P:(f + 1) * P]  # [3072, 128]
        w1p[:, f * CK * P:(f + 1) * CK * P] = (
            blk.reshape(CK, P, P).transpose(1, 0, 2).reshape(P, CK * P)
        )
    w1flat = np.concatenate(
        [w1p[:, f * CK * P:(f + 1) * CK * P].ravel() for f in range(HK)]
    )
    wb = np.concatenate([
        np.asarray(inputs["proj_w"], np.float32).ravel(),
        np.asarray(inputs["gcn_w"], np.float32).ravel(),
        np.asarray(inputs["mlp_w2"], np.float32).ravel(),
        w1flat,
    ]).astype(BF).reshape(1, WBTOT)
    wf = np.concatenate([
        bcast(inputs["proj_b"]),
        bcast(inputs["ln1_g"]),
        bcast(inputs["ln1_b"]),
        bcast(inputs["mlp_b2"]),
        bcast(inputs["ln2_g"]),
        bcast(inputs["ln2_b"]),
        gbcol,
        b1col,
    ], axis=1).astype(np.float32)
    assert wf.shape == (P, FTOT)
    wf16 = wf.view(np.uint16).view(BF)  # raw f32 bytes as bf16 columns

    nch = oh.shape[1] // P
    nech = idx16.shape[2] // 8 if idx16.ndim == 3 else 0
    nech = (idx16.shape[-1] * 16) // P
    XT_OFF, OH_OFF, IDX_OFF, WF_OFF, XCTOT = _xc_layout(nch, nech)

    maps = []
    for c in range(NCORES):
        xcn = np.zeros((NPAD, IN_F), np.float32)
        xcn[:NLOC] = x[c * NLOC:(c + 1) * NLOC]
        # pack: xp[p, m*IN_F + k*128 + n] = x[m*128+n, k*128+p]
        xp = (
            xcn.reshape(MT, P, INK, P)     # [m, n, k, p]
            .transpose(3, 0, 2, 1)          # [p, m, k, n]
            .reshape(P, MT * IN_F)
        )
        ohp = (
            oh[c].astype(np.float32).reshape(-1, P, P)  # [ch, p, d]
            .transpose(1, 0, 2)                          # [p, ch, d]
            .reshape(P, -1)
        )
        xcat = np.zeros((P, XCTOT), BF)
        xcat[:, XT_OFF:XT_OFF + MT * IN_F] = xp.astype(BF)
        xcat[:, OH_OFF:OH_OFF + nch * P] = ohp.astype(BF)
        xcat[:, IDX_OFF:IDX_OFF + (nech * P) // 16] = (
            idx16[c].view(BF)
        )
        xcat[:, WF_OFF:WF_OFF + 2 * FTOT] = wf16
        maps.append({"xc": xcat, "wb": wb})
    return maps


def kernel(**inputs):
    nch, c_list, gidx, oh, idx16, nech = _prep(np.asarray(inputs["edge_index"]))
    nc = _build(nch, c_list, nech, stage=4)
    maps = _in_maps(inputs, nch, gidx, oh, idx16)
    res = run_bass_kernel_spmd(nc, maps, list(range(NCORES)))
    outs = [res.results[c]["out"][:NLOC] for c in range(NCORES)]
    return np.concatenate(outs, axis=0).astype(np.float32)



# revision 26
# speedup vs baseline: 1.4094x; 1.0453x over previous
"""GCN context-paper kernel for 8 trn2 NeuronCores (SPMD via bass/Tile).

Model (see reference): proj+LN -> 3x GCNConv(+self loops, sym-norm) with
GELU -> concat(4 hops) -> MLP(GELU) -> LN.

Sharding: nodes partitioned across 8 cores (2500/core, padded to 2560).
Per hop: each core computes Y = h @ W for its nodes, AllGathers Y (bf16),
then builds its nodes' aggregation with indirect row-gathers of Y plus
one-hot matmuls on the tensor engine (edge weights folded into the
one-hot values); self-loop terms read local Y straight from SBUF.

Layout strategy: activations are kept feature-major ("ct" tiles,
[128 feat, 2560 nodes]) which the scatter matmul produces directly and
all lhsT uses consume directly; only the proj output needs PE transposes.

Perf notes (measured on the axon-tunneled 8-core setup):
- Per-exec dispatch cost through the PJRT tunnel is ~40 us PER OPERAND,
  so all inputs are consolidated into two tensors: per-core `xc`
  (x^T | one-hot | gather idx | f32 tables, bitcast into bf16 columns)
  and shared flat weights `wb`.
- Each collective has a ~120 us fixed cost here, so ONE full-width
  AllGather per hop beats a feature-halved pair (split_ag=False);
  per-tile agin streaming lets it start right after the last Y matmul.
- LN uses var = E[x^2]-mu^2 (square on ACT overlaps mean-reduce on DVE)
  and a fused (x-mu)*rs tensor_scalar.

DMA discipline: every DMA-queue instruction must end up with at most ONE
semaphore wait (hardware struct limit). Hence: DMA destinations in SBUF
are either fresh tiles or have engine-op (not DMA) prior writers; DMA
sources are external inputs or covered by dummy lane-warming DMAs
(collective output).
"""

import numpy as np
import ml_dtypes

import concourse.bass as bass
import concourse.bacc as bacc
import concourse.mybir as mybir
import concourse.tile as tile
from concourse.bass_utils import run_bass_kernel_spmd
from concourse.masks import make_identity

# problem constants (hardcoded per contract)
N, E, IN_F, H, HOPS = 20000, 100000, 1536, 768, 3
LN_EPS = 1e-5
NCORES = 8
NLOC = N // NCORES            # 2500 real nodes per core
P = 128
MT = 20                       # node tiles per core
NPAD = MT * P                 # 2560 padded nodes per core
HK = H // P                   # 6 feature tiles
HH = H // 2                   # feature half width (AllGather split)
HKH = HK // 2                 # feature tiles per half
INK = IN_F // P               # 12
CK = (HOPS + 1) * H // P      # 24 cat feature tiles
NSL = ((0, 512), (512, 256))  # N-dim slices for 768-wide outputs
OB = 8                        # chunks per one-hot load
GRP = 8                       # edge chunks per dma_gather
NWARM = 2                     # dummy lane-warming DMAs per collective

# flat offsets into the consolidated bf16 weight tensor (elements)
PROJ_OFF = 0
GCN_OFF = PROJ_OFF + IN_F * H
W2_OFF = GCN_OFF + HOPS * H * H
W1_OFF = W2_OFF + H * H
WBTOT = W1_OFF + (HOPS + 1) * H * H
# flat offsets into the consolidated f32 table section (per-partition cols)
PB_OFF, L1G_OFF, L1B_OFF = 0, H, 2 * H
B2_OFF, L2G_OFF, L2B_OFF = 3 * H, 4 * H, 5 * H
GB_OFF = 6 * H
B1_OFF = GB_OFF + HOPS * HK
FTOT = B1_OFF + HK


def _pad16(x):
    return (x + 15) // 16 * 16


def _xc_layout(nch, nech):
    """bf16-column offsets of the sections in the per-core xc tensor."""
    xt_off = 0
    oh_off = _pad16(xt_off + MT * IN_F)
    idx_off = _pad16(oh_off + nch * P)
    wf_off = _pad16(idx_off + (nech * P) // 16)
    tot = _pad16(wf_off + 2 * FTOT)
    return xt_off, oh_off, idx_off, wf_off, tot

F32 = mybir.dt.float32
BF16 = mybir.dt.bfloat16
I32 = mybir.dt.int32
BF = ml_dtypes.bfloat16


# ---------------------------------------------------------------- host prep

def _prep(edge_index):
    """Host preprocessing: normalization, edge sorting, per-core chunk
    tables (gather indices + one-hot weight blocks)."""
    src = np.asarray(edge_index[0], dtype=np.int64)
    dst = np.asarray(edge_index[1], dtype=np.int64)
    deg = np.bincount(dst, minlength=N).astype(np.float64) + 1.0
    dis = 1.0 / np.sqrt(deg)

    # real edges only; self loops become a dedicated per-tile identity chunk
    alls, alld = src, dst
    w = (dis[alls] * dis[alld]).astype(np.float32)

    # global row in the AllGather output for each source node
    yg_row = (alls // NLOC) * NPAD + (alls % NLOC)

    # group edges by (core, dst tile)
    core = alld // NLOC
    loc = alld % NLOC
    t = loc // P
    d = loc % P  # local offset within dst tile
    counts = np.zeros((NCORES, MT), dtype=np.int64)
    np.add.at(counts, (core, t), 1)
    # chunk 0 of each tile = self loops (plain DMA from local Y); rest edges
    c_list = [1 + max(1, int(np.ceil(counts[:, tt].max() / P))) for tt in range(MT)]
    off = np.zeros(MT, dtype=np.int64)
    off[1:] = np.cumsum(c_list)[:-1]
    nch = int(sum(c_list))

    gidx = np.zeros((NCORES, P, nch), dtype=np.int32)
    oh = np.zeros((NCORES, nch * P, P), dtype=np.float32)

    # self chunks: diag(dis[d]^2) per (core, tile)
    for cc in range(NCORES):
        for tt in range(MT):
            nreal = min(P, NLOC - tt * P)
            gl = cc * NLOC + tt * P + np.arange(nreal)
            ch = off[tt]
            oh[cc, ch * P + np.arange(nreal), np.arange(nreal)] = (
                dis[gl] * dis[gl]
            )

    order = np.lexsort((alls, t, core))  # stable ordering by (core, tile)
    so_core, so_t, so_d = core[order], t[order], d[order]
    so_w, so_yg = w[order], yg_row[order]
    grp = so_core * MT + so_t
    start = np.zeros(NCORES * MT + 1, dtype=np.int64)
    np.add.at(start, grp + 1, 1)
    start = np.cumsum(start)
    pos = np.arange(len(order)) - start[grp]
    chunk = off[so_t] + 1 + pos // P
    row = pos % P
    gidx[so_core, row, chunk] = so_yg.astype(np.int32)
    oh[so_core, chunk * P + row, so_d] = so_w

    # int16 index stream for dma_gather: edge-chunk ids exclude self chunks
    n_self_before = np.zeros(nch, dtype=np.int64)
    for tt in range(MT):
        n_self_before[off[tt]:] += 0  # placeholder
    # chunk -> edge-chunk id: subtract #self chunks with index <= chunk
    selfmask = np.zeros(nch, dtype=np.int64)
    selfmask[off] = 1
    ech_of = np.cumsum(selfmask) - 1  # for self chunks: id of tile
    ech_map = np.arange(nch) - np.cumsum(selfmask)  # edge-chunk id (c>0)
    nech = nch - MT
    ni_tot = nech * P
    idx16 = np.zeros((NCORES, 128, ni_tot // 16), dtype=np.int16)
    e_ch = ech_map[chunk]  # edge-chunk id per sorted edge
    i_flat = e_ch * P + row
    p16 = i_flat % 16
    c16 = i_flat // 16
    for cc in range(NCORES):
        m = so_core == cc
        a = np.zeros((16, ni_tot // 16), np.int16)
        a[p16[m], c16[m]] = so_yg[m].astype(np.int16)
        idx16[cc] = np.tile(a, (8, 1))
    return nch, c_list, gidx, oh.astype(BF), idx16, nech


# --------------------------------------------------------------- bass build

def _build(nch, c_list, nech, stage=4, fake_ag=False, split_ag=False):
    """Emit the SPMD Bass program. stage: 1=proj only, 2=+1 hop,
    3=+3 hops, 4=full (MLP+LN2). For stage<4 the output is the ct
    (feature-major) tiles of the last computed hop, [768, NPAD] f32."""
    nc = bacc.Bacc(
        "TRN2", target_bir_lowering=False, debug=False, num_devices=NCORES,
        num_swdge_queues=4,
    )
    dp = nc.declare_dram_parameter
    # consolidated operands: fewer kernel params -> much lower per-exec
    # dispatch cost through the PJRT tunnel (~40 us/operand measured).
    # xc = [x^T | one-hot tables | gather indices (bitcast) | f32 tables
    # (bitcast)], all per-core; wb = shared flat bf16 weights.
    XT_OFF, OH_OFF, IDX_OFF, WF_OFF, XCTOT = _xc_layout(nch, nech)
    xc = dp("xc", [P, XCTOT], BF16, isOutput=False)
    wb = dp("wb", [1, WBTOT], BF16, isOutput=False)

    def wb2d(off, rows, cols):
        # contiguous [rows, cols] block at flat offset `off` in wb
        return wb[0:1, off:off + rows * cols].rearrange(
            "o (p h) -> (o p) h", p=rows
        )

    def wfap(off, width):
        # f32 view into the per-core xc tensor
        return xc[:, WF_OFF + 2 * off:WF_OFF + 2 * (off + width)].bitcast(F32)

    nhop = 0 if stage <= 1 else (1 if stage == 2 else HOPS)
    if stage >= 4:
        out = dp("out", [NPAD, H], F32, isOutput=True)
    else:
        out = dp("out", [H, NPAD], F32, isOutput=True)

    off = np.zeros(MT, dtype=np.int64)
    off[1:] = np.cumsum(c_list)[:-1]

    with tile.TileContext(nc) as tc:
        import contextlib

        with contextlib.ExitStack() as ctx:
            dram = ctx.enter_context(tc.tile_pool(name="dram", bufs=1, space="DRAM"))
            cat = ctx.enter_context(tc.tile_pool(name="cat", bufs=1))
            cst = ctx.enter_context(tc.tile_pool(name="cst", bufs=1))

            # persistent feature-major activation tiles
            ct = [cat.tile([P, NPAD], BF16, name=f"ct{i}") for i in range(CK)]
            # persistent slabs for 4 of the 6 w1 blocks (loaded after proj
            # so the MLP doesn't stall on its weight stream); only fits
            # when the halved gather tiles free enough SBUF
            npre = 4 if split_ag else 0
            w1pre = [cat.tile([P, CK, P], BF16, name=f"w1pre{f}")
                     for f in range(npre)]

            idx_sb = cst.tile([128, (nech * P) // 16], mybir.dt.int16)
            gb_sb = cst.tile([P, HOPS * HK], F32)
            ident = cst.tile([P, P], BF16)
            make_identity(nc, ident[:])
            eps_t = cst.tile([P, 1], F32)
            nc.gpsimd.memset(eps_t[:], LN_EPS)

            # ---------------- proj + LN1 -> ct[0..5] (via PE transpose)
            with tc.tile_pool(name="proj", bufs=1) as pp, \
                    tc.tile_pool(name="psum_pj", bufs=1, space="PSUM") as psum:
                # first x slab first so the PE can start ASAP
                xs0 = pp.tile([P, INK, P], BF16, tag="xslab", bufs=3)
                nc.sync.dma_start(
                    out=xs0[:],
                    in_=xc[:, XT_OFF:XT_OFF + IN_F].rearrange(
                        "p (k n) -> p k n", n=P
                    ),
                )
                pw = [pp.tile([P, H], BF16, name=f"pw{k}") for k in range(INK)]
                for k in range(INK):
                    nc.sync.dma_start(
                        out=pw[k][:], in_=wb2d(PROJ_OFF + k * P * H, P, H)
                    )
                lt1 = pp.tile([P, 3 * H], F32)
                nc.sync.dma_start(out=lt1[:], in_=wfap(PB_OFF, 3 * H))
                pb_sb = lt1[:, 0:H]
                l1g_sb = lt1[:, H:2 * H]
                l1b_sb = lt1[:, 2 * H:3 * H]

                for m in range(MT):
                    ms = slice(m * P, (m + 1) * P)
                    if m == 0:
                        xs = xs0
                    else:
                        xs = pp.tile([P, INK, P], BF16, tag="xslab", bufs=3)
                        nc.sync.dma_start(
                            out=xs[:],
                            in_=xc[:, XT_OFF + m * IN_F:
                                   XT_OFF + (m + 1) * IN_F].rearrange(
                                "p (k n) -> p k n", n=P
                            ),
                        )
                    ps = psum.tile([P, H], F32, tag="pj", bufs=2)
                    for n0, nn in NSL:
                        for k in range(INK):
                            nc.tensor.matmul(
                                out=ps[:, n0:n0 + nn],
                                lhsT=xs[:, k, :],
                                rhs=pw[k][:, n0:n0 + nn],
                                start=(k == 0),
                                stop=(k == INK - 1),
                            )
                    # LN1 over features (free dim), node-major.
                    # var = E[x^2] - mu^2 so the square (on ACT) overlaps the
                    # mean reduce (on DVE); Rsqrt + fused (x-mu)*rs cut DVE ops.
                    t0 = pp.tile([P, H], F32, tag="t0", bufs=2)
                    nc.vector.tensor_add(out=t0[:], in0=ps[:], in1=pb_sb)
                    mu = pp.tile([P, 1], F32, tag="mu", bufs=2)
                    nc.vector.reduce_sum(out=mu[:], in_=t0[:], axis=mybir.AxisListType.X)
                    sq = pp.tile([P, H], F32, tag="sq", bufs=2)
                    nc.scalar.activation(
                        out=sq[:], in_=t0[:],
                        func=mybir.ActivationFunctionType.Square,
                    )
                    var = pp.tile([P, 1], F32, tag="var", bufs=2)
                    nc.vector.reduce_sum(out=var[:], in_=sq[:], axis=mybir.AxisListType.X)
                    nc.scalar.mul(out=mu[:], in_=mu[:], mul=1.0 / H)
                    m2 = pp.tile([P, 1], F32, tag="m2", bufs=2)
                    nc.vector.tensor_mul(out=m2[:], in0=mu[:], in1=mu[:])
                    nc.vector.tensor_scalar(
                        out=var[:], in0=var[:], scalar1=1.0 / H, scalar2=None,
                        op0=mybir.AluOpType.mult,
                    )
                    nc.vector.tensor_sub(out=var[:], in0=var[:], in1=m2[:])
                    rs = pp.tile([P, 1], F32, tag="rs", bufs=2)
                    nc.scalar.activation(
                        out=rs[:], in_=var[:],
                        func=mybir.ActivationFunctionType.Sqrt,
                        bias=eps_t[:, :1],
                    )
                    nc.vector.reciprocal(out=rs[:], in_=rs[:])
                    nc.vector.tensor_scalar(
                        out=t0[:], in0=t0[:], scalar1=mu[:, :1], scalar2=rs[:, :1],
                        op0=mybir.AluOpType.subtract, op1=mybir.AluOpType.mult,
                    )
                    nc.vector.tensor_mul(out=t0[:], in0=t0[:], in1=l1g_sb)
                    h0 = pp.tile([P, H], BF16, tag="h0", bufs=2)
                    nc.vector.tensor_add(out=h0[:], in0=t0[:], in1=l1b_sb)
                    # transpose 6 blocks -> ct[f][:, m]
                    for f in range(HK):
                        tp = psum.tile([P, P], BF16, tag="tp", bufs=2)
                        nc.tensor.transpose(
                            out=tp[:], in_=h0[:, f * P:(f + 1) * P], identity=ident[:]
                        )
                        nc.scalar.activation(
                            out=ct[f][:, ms], in_=tp[:],
                            func=mybir.ActivationFunctionType.Copy,
                        )

            # deferred loads: not needed until the first scatter pass / MLP,
            # so keep them off the critical startup DMA queue
            nc.sync.dma_start(
                out=idx_sb[:],
                in_=xc[:, IDX_OFF:IDX_OFF + (nech * P) // 16].bitcast(
                    mybir.dt.int16
                ),
            )
            nc.sync.dma_start(out=gb_sb[:], in_=wfap(GB_OFF, HOPS * HK))
            for f in range(npre):
                nc.sync.dma_start(
                    out=w1pre[f][:],
                    in_=wb[0:1, W1_OFF + f * P * CK * P:
                           W1_OFF + (f + 1) * P * CK * P].rearrange(
                        "o (p k n) -> (o p) k n", p=P, n=P
                    ),
                )

            # ---------------- hops
            for k in range(nhop):
                hp = tc.tile_pool(name=f"hop{k}", bufs=1)
                with hp as hpool, \
                        tc.tile_pool(name=f"psum_h{k}", bufs=1, space="PSUM") as psum:
                    gw = [hpool.tile([P, H], BF16, name=f"gw{k}_{f}") for f in range(HK)]
                    for f in range(HK):
                        nc.sync.dma_start(
                            out=gw[f][:],
                            in_=wb2d(GCN_OFF + (k * H + f * P) * H, P, H),
                        )
                    ybig = hpool.tile([P, MT * H], BF16)
                    # feature-halved AllGather: gather/scatter of half 0
                    # overlaps the collective for half 1 (split_ag=True);
                    # otherwise one full-width AllGather
                    nhalf = 2 if split_ag else 1
                    hw_ = HH if split_ag else H
                    agins = [
                        dram.tile([NPAD, hw_], BF16, name=f"agin{k}_{h}")
                        for h in range(nhalf)
                    ]
                    ygs = [
                        dram.tile([NCORES * NPAD, hw_], BF16, addr_space="Shared",
                                  name=f"yg{k}_{h}")
                        for h in range(nhalf)
                    ]
                    for m in range(MT):
                        ms = slice(m * P, (m + 1) * P)
                        ps = psum.tile([P, H], F32, tag="y", bufs=2)
                        for n0, nn in NSL:
                            for f in range(HK):
                                nc.tensor.matmul(
                                    out=ps[:, n0:n0 + nn],
                                    lhsT=ct[6 * k + f][:, ms],
                                    rhs=gw[f][:, n0:n0 + nn],
                                    start=(f == 0),
                                    stop=(f == HK - 1),
                                )
                        nc.scalar.activation(
                            out=ybig[:, m * H:(m + 1) * H], in_=ps[:],
                            func=mybir.ActivationFunctionType.Copy,
                        )
                        # stream this tile's rows to DRAM immediately so the
                        # AllGather can start right after the last Y matmul
                        nc.sync.dma_start(
                            out=agins[0][m * P:(m + 1) * P, :],
                            in_=ybig[:, m * H:m * H + hw_],
                        )
                        if split_ag:
                            nc.scalar.dma_start(
                                out=agins[1][m * P:(m + 1) * P, :],
                                in_=ybig[:, m * H + HH:(m + 1) * H],
                            )
                    for h in range(nhalf):
                        if fake_ag:
                            # timing-proxy only: local copy standing in for the
                            # AllGather (the sim's collective model is ~12x
                            # pessimistic for intra-chip groups)
                            nc.gpsimd.dma_start(out=ygs[h][0:NPAD, :], in_=agins[h][:])
                        else:
                            nc.gpsimd.collective_compute(
                                "AllGather",
                                mybir.AluOpType.bypass,
                                ins=[agins[h].opt()],
                                outs=[ygs[h].opt()],
                                replica_groups=[list(range(NCORES))],
                            )

                    # scatter passes, one per feature half
                    nch_tot = int(sum(c_list))
                    for half in range(nhalf):
                        yg = ygs[half]
                        # warm SWDGE lanes with 1-dep dummy reads of yg
                        for dlane in range(NWARM):
                            dmy = hpool.tile([2, 4], BF16,
                                             tag=f"dmy{half}_{dlane}", bufs=1)
                            nc.gpsimd.dma_start(
                                out=dmy[:], in_=yg[dlane * 2:dlane * 2 + 2, 0:4]
                            )
                        if k == 0 and half == 0:
                            # warm lanes on the idx region too (SBUF->SBUF tiny)
                            for dlane in range(NWARM):
                                dmi = hpool.tile([2, 1], I32, tag=f"dmi{dlane}",
                                                 bufs=1)
                                nc.gpsimd.dma_start(
                                    out=dmi[:], in_=idx_sb[dlane:dlane + 2, 0:1]
                                )
                        oh_tiles = {}
                        g_tiles = {}
                        ech = 0  # running edge-chunk id
                        for t in range(MT):
                            ts = slice(t * P, (t + 1) * P)
                            npa = HKH if split_ag else HK
                            if split_ag:
                                pa = psum.tile([P, HKH * P], F32,
                                               tag=f"sc{half}", bufs=2)
                                pb_ = None
                            else:
                                pa = psum.tile([P, 512], F32, tag="sca", bufs=2)
                                pb_ = psum.tile([P, 256], F32, tag="scb", bufs=2)

                            def _dst(fi):
                                if pb_ is None or fi < 4:
                                    return pa[:, (fi % 4) * P:(fi % 4 + 1) * P]
                                return pb_[:, (fi - 4) * P:(fi - 3) * P]

                            starts = (0,) if pb_ is None else (0, 4)
                            stops = (npa - 1,) if pb_ is None else (3, 5)
                            for c in range(c_list[t]):
                                ch = int(off[t]) + c
                                if c == 0:
                                    # self-loop chunk: local Y rows already in
                                    # SBUF node-major (ybig)
                                    gsl = ybig[:, t * H + half * HH:
                                               t * H + half * HH + hw_]
                                else:
                                    gg, gj = ech // GRP, ech % GRP
                                    if gj == 0:
                                        ng = min(GRP, nech - gg * GRP)
                                        gt = hpool.tile(
                                            [P, ng, hw_], BF16, tag="g",
                                            bufs=2, name=f"g{k}_{half}_{gg}",
                                        )
                                        nc.gpsimd.dma_gather(
                                            out_ap=gt[:],
                                            in_ap=yg[:],
                                            idxs_ap=idx_sb[
                                                :, gg * GRP * 8:(gg * GRP + ng) * 8
                                            ],
                                            num_idxs=ng * P,
                                            num_idxs_reg=ng * P,
                                            elem_size=hw_,
                                            queue_num=gg % 4,
                                        )
                                        g_tiles[gg] = gt
                                    gsl = g_tiles[gg][:, gj, :]
                                    ech += 1
                                og, oj = ch // OB, ch % OB
                                if oj == 0:
                                    no = min(OB, nch_tot - og * OB)
                                    oh_t = hpool.tile(
                                        [P, no, P], BF16, tag="oh", bufs=3,
                                        name=f"oh{k}_{half}_{og}",
                                    )
                                    nc.sync.dma_start(
                                        out=oh_t[:],
                                        in_=xc[
                                            :, OH_OFF + og * OB * P:
                                            OH_OFF + (og * OB + no) * P
                                        ].rearrange("p (c m) -> p c m", m=P),
                                    )
                                    oh_tiles[og] = oh_t
                                oh_t = oh_tiles[og]
                                first, last = (c == 0), (c == c_list[t] - 1)
                                for fi in range(npa):
                                    nc.tensor.matmul(
                                        out=_dst(fi),
                                        lhsT=gsl[:, fi * P:(fi + 1) * P],
                                        rhs=oh_t[:, oj, :],
                                        start=first and fi in starts,
                                        stop=last and fi in stops,
                                    )
                            for fi in range(npa):
                                f = half * HKH + fi
                                nc.scalar.activation(
                                    out=ct[6 * (k + 1) + f][:, ts],
                                    in_=_dst(fi),
                                    func=mybir.ActivationFunctionType.Gelu,
                                    bias=gb_sb[:, k * HK + f:k * HK + f + 1],
                                )

            if stage < 4:
                # dump last hop's ct tiles as [H, NPAD] f32
                with tc.tile_pool(name="dump", bufs=1) as dpool:
                    for f in range(HK):
                        df = dpool.tile([P, NPAD], F32, tag="df", bufs=2)
                        nc.vector.tensor_copy(out=df[:], in_=ct[6 * nhop + f][:])
                        nc.sync.dma_start(out=out[f * P:(f + 1) * P, :], in_=df[:])

            if stage >= 4:
                # ---------------- MLP + LN2
                with tc.tile_pool(name="mlp", bufs=1) as mp, \
                        tc.tile_pool(name="psum_mlp", bufs=1, space="PSUM") as psum:
                    w2t = [mp.tile([P, H], BF16, name=f"w2t{f}") for f in range(HK)]
                    for f in range(HK):
                        nc.sync.dma_start(
                            out=w2t[f][:], in_=wb2d(W2_OFF + f * P * H, P, H)
                        )
                    b1_sb = mp.tile([P, HK], F32)
                    nc.sync.dma_start(out=b1_sb[:], in_=wfap(B1_OFF, HK))
                    lt2 = mp.tile([P, 3 * H], F32)
                    nc.sync.dma_start(out=lt2[:], in_=wfap(B2_OFF, 3 * H))
                    b2_sb = lt2[:, 0:H]
                    l2g_sb = lt2[:, H:2 * H]
                    l2b_sb = lt2[:, 2 * H:3 * H]
                    # w1: blocks 0-3 were preloaded persistently after proj;
                    # blocks 4-5 stream here, overlapped with the first
                    # chunk's matmuls on blocks 0-3
                    w1sb = w1pre + [
                        mp.tile([P, CK, P], BF16, name=f"w1sb{f}")
                        for f in range(npre, HK)
                    ]
                    for n in range(5):  # 512-wide node chunks
                        ns = slice(n * 512, (n + 1) * 512)
                        zt = [
                            mp.tile([P, 512], BF16, tag=f"zt{f}", bufs=2, name=f"zt{f}")
                            for f in range(HK)
                        ]
                        for f in range(HK):
                            if n == 0 and f >= npre:
                                nc.sync.dma_start(
                                    out=w1sb[f][:],
                                    in_=wb[0:1, W1_OFF + f * P * CK * P:
                                           W1_OFF + (f + 1) * P * CK * P].rearrange(
                                        "o (p k n) -> (o p) k n", p=P, n=P
                                    ),
                                )
                            pz = psum.tile([P, 512], F32, tag="z", bufs=2)
                            for kk in range(CK):
                                nc.tensor.matmul(
                                    out=pz[:],
                                    lhsT=w1sb[f][:, kk, :],
                                    rhs=ct[kk][:, ns],
                                    start=(kk == 0),
                                    stop=(kk == CK - 1),
                                )
                            nc.scalar.activation(
                                out=zt[f][:],
                                in_=pz[:],
                                func=mybir.ActivationFunctionType.Gelu,
                                bias=b1_sb[:, f:f + 1],
                            )
                        for mm in range(4):
                            m = n * 4 + mm
                            po = psum.tile([P, H], F32, tag="o", bufs=2)
                            for n0, nn in NSL:
                                for f in range(HK):
                                    nc.tensor.matmul(
                                        out=po[:, n0:n0 + nn],
                                        lhsT=zt[f][:, mm * P:(mm + 1) * P],
                                        rhs=w2t[f][:, n0:n0 + nn],
                                        start=(f == 0),
                                        stop=(f == HK - 1),
                                    )
                            t0 = mp.tile([P, H], F32, tag="t0", bufs=2)
                            nc.vector.tensor_add(out=t0[:], in0=po[:], in1=b2_sb)
                            mu = mp.tile([P, 1], F32, tag="mu", bufs=2)
                            nc.vector.reduce_sum(
                                out=mu[:], in_=t0[:], axis=mybir.AxisListType.X
                            )
                            sq = mp.tile([P, H], F32, tag="sq", bufs=2)
                            nc.scalar.activation(
                                out=sq[:], in_=t0[:],
                                func=mybir.ActivationFunctionType.Square,
                            )
                            var = mp.tile([P, 1], F32, tag="var", bufs=2)
                            nc.vector.reduce_sum(
                                out=var[:], in_=sq[:], axis=mybir.AxisListType.X
                            )
                            nc.scalar.mul(out=mu[:], in_=mu[:], mul=1.0 / H)
                            m2 = mp.tile([P, 1], F32, tag="m2", bufs=2)
                            nc.vector.tensor_mul(out=m2[:], in0=mu[:], in1=mu[:])
                            nc.vector.tensor_scalar(
                                out=var[:], in0=var[:], scalar1=1.0 / H,
                                scalar2=None, op0=mybir.AluOpType.mult,
                            )
                            nc.vector.tensor_sub(out=var[:], in0=var[:], in1=m2[:])
                            rs = mp.tile([P, 1], F32, tag="rs", bufs=2)
                            nc.scalar.activation(
                                out=rs[:], in_=var[:],
                                func=mybir.ActivationFunctionType.Sqrt,
                                bias=eps_t[:, :1],
                            )
                            nc.vector.reciprocal(out=rs[:], in_=rs[:])
                            nc.vector.tensor_scalar(
                                out=t0[:], in0=t0[:], scalar1=mu[:, :1],
                                scalar2=rs[:, :1],
                                op0=mybir.AluOpType.subtract,
                                op1=mybir.AluOpType.mult,
                            )
                            nc.vector.tensor_mul(out=t0[:], in0=t0[:], in1=l2g_sb)
                            ot = mp.tile([P, H], F32, tag="ot", bufs=2)
                            nc.vector.tensor_add(out=ot[:], in0=t0[:], in1=l2b_sb)
                            nc.sync.dma_start(out=out[m * P:(m + 1) * P, :], in_=ot[:])
    nc.compile()
    return nc


def check_waits(nc, limit=1):
    """Return list of DMA-queue instructions exceeding the wait limit."""
    bad = []
    for f in nc.m.functions:
        for bb in f.blocks:
            for ins in bb.instructions:
                tn = type(ins).__name__
                if tn not in ("InstDMACopy", "InstDmaTransposeAnt"):
                    continue
                si = ins.sync_info
                if len(si.on_wait) > limit:
                    bad.append(
                        (ins.name, tn, str(ins.engine),
                         [(w.ant_name, w.wait_value) for w in si.on_wait])
                    )
    return bad


# ------------------------------------------------------------- entry point

def _in_maps(inputs, nch, gidx, oh, idx16):
    x = np.asarray(inputs["x"], dtype=np.float32)
    bcast = lambda v: np.broadcast_to(
        np.asarray(v, np.float32), (P, H)
    ).copy()
    gb = np.asarray(inputs["gcn_b"], np.float32)  # [HOPS, H]
    gbcol = np.zeros((P, HOPS * HK), np.float32)
    for k in range(HOPS):
        for f in range(HK):
            gbcol[:, k * HK + f] = gb[k, f * P:(f + 1) * P]
    b1 = np.asarray(inputs["mlp_b1"], np.float32)
    b1col = np.zeros((P, HK), np.float32)
    for f in range(HK):
        b1col[:, f] = b1[f * P:(f + 1) * P]
    w1 = np.asarray(inputs["mlp_w1"], np.float32)  # [3072, 768]
    w1p = np.zeros((P, HK * (HOPS + 1) * H), np.float32)
    for f in range(HK):
        blk = w1[:, f * P:(f + 1) * P]  # [3072, 128]
        w1p[:, f * CK * P:(f + 1) * CK * P] = (
            blk.reshape(CK, P, P).transpose(1, 0, 2).reshape(P, CK * P)
        )
    w1flat = np.concatenate(
        [w1p[:, f * CK * P:(f + 1) * CK * P].ravel() for f in range(HK)]
    )
    wb = np.concatenate([
        np.asarray(inputs["proj_w"], np.float32).ravel(),
        np.asarray(inputs["gcn_w"], np.float32).ravel(),
        np.asarray(inputs["mlp_w2"], np.float32).ravel(),
        w1flat,
    ]).astype(BF).reshape(1, WBTOT)
    wf = np.concatenate([
        bcast(inputs["proj_b"]),
        bcast(inputs["ln1_g"]),
        bcast(inputs["ln1_b"]),
        bcast(inputs["mlp_b2"]),
        bcast(inputs["ln2_g"]),
        bcast(inputs["ln2_b"]),
        gbcol,
        b1col,
    ], axis=1).astype(np.float32)
    assert wf.shape == (P, FTOT)
    wf16 = wf.view(np.uint16).view(BF)  # raw f32 bytes as bf16 columns

    nch = oh.shape[1] // P
    nech = idx16.shape[2] // 8 if idx16.ndim == 3 else 0
    nech = (idx16.shape[-1] * 16) // P
    XT_OFF, OH_OFF, IDX_OFF, WF_OFF, XCTOT = _xc_layout(nch, nech)

    maps = []
    for c in range(NCORES):
        xcn = np.zeros((NPAD, IN_F), np.float32)
        xcn[:NLOC] = x[c * NLOC:(c + 1) * NLOC]
        # pack: xp[p, m*IN_F + k*128 + n] = x[m*128+n, k*128+p]
        xp = (
            xcn.reshape(MT, P, INK, P)     # [m, n, k, p]
            .transpose(3, 0, 2, 1)          # [p, m, k, n]
            .reshape(P, MT * IN_F)
        )
        ohp = (
            oh[c].astype(np.float32).reshape(-1, P, P)  # [ch, p, d]
            .transpose(1, 0, 2)                          # [p, ch, d]
            .reshape(P, -1)
        )
        xcat = np.zeros((P, XCTOT), BF)
        xcat[:, XT_OFF:XT_OFF + MT * IN_F] = xp.astype(BF)
        xcat[:, OH_OFF:OH_OFF + nch * P] = ohp.astype(BF)
        xcat[:, IDX_OFF:IDX_OFF + (nech * P) // 16] = (
            idx16[c].view(BF)
        )
        xcat[:, WF_OFF:WF_OFF + 2 * FTOT] = wf16
        maps.append({"xc": xcat, "wb": wb})
    return maps


def kernel(**inputs):
    nch, c_list, gidx, oh, idx16, nech = _prep(np.asarray(inputs["edge_index"]))
    nc = _build(nch, c_list, nech, stage=4)
    maps = _in_maps(inputs, nch, gidx, oh, idx16)
    res = run_bass_kernel_spmd(nc, maps, list(range(NCORES)))
    outs = [res.results[c]["out"][:NLOC] for c in range(NCORES)]
    return np.concatenate(outs, axis=0).astype(np.float32)

